# revision 1
# baseline (speedup 1.0000x reference)
"""MHSA + RoPE kernel for Trainium2, 8 NeuronCores.

Sharding: data-parallel over batch (B=2) x tensor-parallel over heads
(16 heads -> 4 head-groups of 4). Core c handles batch c//4, heads
[4*(c%4) : 4*(c%4)+4]. Each core computes its partial o_proj output
[N, D]; host sums the 4 partials per batch (the "all-reduce").

Device-side layout choices (per core):
  - q/k are computed directly in transposed layout qT/kT [d, n] so the
    scores matmul S^T[j,i] = k_j . q_i needs no transposes, and the PV
    matmul produces A^T [d, i] which is exactly the lhsT layout o_proj
    needs.
  - softmax denominators: S^T tiles are exp'd on ScalarE, accumulated
    elementwise over j-tiles on VectorE, then partition-reduced with a
    ones-vector matmul; reciprocal is broadcast back over partitions
    with a ones-row matmul.
  - projections and o_proj run in bf16 (inputs pre-cast on host);
    scores / PV run in float32r (full-rate fp32 mode of the PE).
"""

import sys

sys.path.insert(0, "/opt/trn_rl_repo")

import numpy as np
import ml_dtypes

import concourse.bass as bass
import concourse.tile as tile
from concourse import bacc, mybir
from concourse.bass_utils import run_bass_kernel_spmd

F32 = mybir.dt.float32
F32R = mybir.dt.float32r
BF16 = mybir.dt.bfloat16
MULT = mybir.AluOpType.mult
ADD = mybir.AluOpType.add
EXP = mybir.ActivationFunctionType.Exp
PSUM = bass.MemorySpace.PSUM

B, N, D = 2, 2048, 2048
H, HD = 16, 128
HL = 4            # local heads per core
C = HL * HD       # 512 local head cols
KT = D // 128     # 16 contraction tiles
NB = 4            # n-blocks of 512 for projections
NT = N // 128     # 16 j-tiles
SCALE = float(HD) ** -0.5
N_CORES = 8

_CACHE = {}


def _build_program():
    nc = bacc.Bacc("TRN2", target_bir_lowering=False, debug=False,
                   num_devices=N_CORES)

    xt_d = nc.dram_tensor("xt", [NB, 128, KT, 512], BF16, kind="ExternalInput")
    wq_d = nc.dram_tensor("wq", [128, KT, C], BF16, kind="ExternalInput")
    wk_d = nc.dram_tensor("wk", [128, KT, C], BF16, kind="ExternalInput")
    wv_d = nc.dram_tensor("wv", [128, KT, C], BF16, kind="ExternalInput")
    wo_d = nc.dram_tensor("wo", [128, HL, D], BF16, kind="ExternalInput")
    cos_d = nc.dram_tensor("cos", [128, N], F32R, kind="ExternalInput")
    sin_d = nc.dram_tensor("sin", [128, N], F32R, kind="ExternalInput")
    onec_d = nc.dram_tensor("onec", [128, 1], F32R, kind="ExternalInput")
    oner_d = nc.dram_tensor("oner", [1, 128], F32R, kind="ExternalInput")
    out_d = nc.dram_tensor("out", [N, D], F32, kind="ExternalOutput")

    with tile.TileContext(nc) as tc:
        with tc.tile_pool(name="res", bufs=1) as res:
            qr = res.tile([128, HL, N], F32R)    # q^T per head [d, n]
            kr = res.tile([128, HL, N], F32R)    # k^T per head [d, n]
            vv = res.tile([128, NT, C], F32R)    # v natural [n, c]
            ones_col = res.tile([128, 1], F32R)
            ones_row = res.tile([1, 128], F32R)
            nc.sync.dma_start(ones_col[:], onec_d[:])
            nc.sync.dma_start(ones_row[:], oner_d[:])

            # ---------------- Phase 1: Q/K/V projections (bf16) ---------
            with (
                tc.tile_pool(name="p1", bufs=1) as p1,
                tc.tile_pool(name="ps1", bufs=6, space=PSUM) as ps1,
            ):
                w_sbs = []
                for wd, wname in ((wq_d, "wq"), (wk_d, "wk"), (wv_d, "wv")):
                    w_sb = p1.tile([128, KT, C], BF16, tag=f"w_{wname}")
                    nc.sync.dma_start(w_sb[:], wd[:])
                    w_sbs.append(w_sb)

                for nb in range(NB):
                    x_sb = p1.tile([128, KT, 512], BF16, tag="x")
                    nc.sync.dma_start(x_sb[:], xt_d[nb])
                    nsl = bass.ts(nb, 512)
                    # q and k projections: psum [d(128) x n(512)] per head
                    for pi, (w_sb, dst) in enumerate(
                        ((w_sbs[0], qr), (w_sbs[1], kr))
                    ):
                        for m in range(HL):
                            ps = ps1.tile([128, 512], F32, tag="ps")
                            for t in range(KT):
                                nc.tensor.matmul(
                                    ps[:],
                                    w_sb[:, t, bass.ts(m, 128)],
                                    x_sb[:, t, :],
                                    start=(t == 0),
                                    stop=(t == KT - 1),
                                )
                            nc.scalar.copy(dst[:, m, nsl], ps[:])
                    # v projection: psum [n(128) x c(512)] per n-chunk
                    for m in range(HL):
                        ps = ps1.tile([128, 512], F32, tag="ps")
                        for t in range(KT):
                            nc.tensor.matmul(
                                ps[:],
                                x_sb[:, t, bass.ts(m, 128)],
                                w_sbs[2][:, t, :],
                                start=(t == 0),
                                stop=(t == KT - 1),
                            )
                        nc.scalar.copy(vv[:, nb * HL + m, :], ps[:])

            # ---------------- Phase 2: RoPE + attention -----------------
            with tc.tile_pool(name="aop", bufs=1) as aop:
                ao = aop.tile([128, HL, N], BF16)   # A^T normalized [c, n]

                with (
                    tc.tile_pool(name="p2", bufs=1) as p2,
                    tc.tile_pool(name="ps_s", bufs=2, space=PSUM) as ps_s,
                    tc.tile_pool(name="ps_a", bufs=1, space=PSUM) as ps_a,
                ):
                    cos_sb = p2.tile([128, N], F32R, tag="cos")
                    sin_sb = p2.tile([128, N], F32R, tag="sin")
                    nc.sync.dma_start(cos_sb[:], cos_d[:])
                    nc.sync.dma_start(sin_sb[:], sin_d[:])

                    # RoPE, in place:  t = shift(q) * sin_signed;
                    # q *= cos; q += t   (sign of sin folded in on host).
                    # The d-half swap is a partition shuffle - compute
                    # engines can't shift partitions, so do it with an
                    # SBUF->SBUF DMA.
                    for src in (qr, kr):
                        for h in range(HL):
                            sl = src[:, h, :]
                            tmp = p2.tile([128, N], F32R, tag="tmp")
                            nc.sync.dma_start(tmp[0:64, :], sl[64:128, :])
                            nc.sync.dma_start(tmp[64:128, :], sl[0:64, :])
                            nc.vector.tensor_tensor(tmp[:], tmp[:], sin_sb[:],
                                                    op=MULT)
                            nc.vector.tensor_tensor(sl, sl, cos_sb[:], op=MULT)
                            nc.vector.tensor_tensor(sl, sl, tmp[:], op=ADD)

                    for h in range(HL):
                        a_ps = ps_a.tile([128, N], F32, tag="a")
                        acc = p2.tile([128, N], F32R, tag="acc")
                        for ih in range(2):
                            ihsl = bass.ts(ih, 1024)
                            for j in range(NT):
                                s_ps = ps_s.tile([128, 1024], F32, tag="s")
                                for f in range(2):
                                    nc.tensor.matmul(
                                        s_ps[:, bass.ts(f, 512)],
                                        kr[:, h, bass.ts(j, 128)],
                                        qr[:, h, ih * 1024 + f * 512 : ih * 1024 + (f + 1) * 512],
                                        start=True, stop=True,
                                    )
                                s_exp = p2.tile([128, 1024], F32R, tag="sexp")
                                nc.scalar.activation(s_exp[:], s_ps[:], EXP,
                                                     scale=SCALE)
                                if j == 0:
                                    nc.vector.tensor_copy(acc[:, ihsl], s_exp[:])
                                else:
                                    nc.vector.tensor_tensor(
                                        acc[:, ihsl], acc[:, ihsl], s_exp[:],
                                        op=ADD)
                                for f in range(2):
                                    nc.tensor.matmul(
                                        a_ps[:, ih * 1024 + f * 512 : ih * 1024 + (f + 1) * 512],
                                        vv[:, j, bass.ts(h, 128)],
                                        s_exp[:, bass.ts(f, 512)],
                                        start=(j == 0), stop=(j == NT - 1),
                                    )
                        # softmax denominators: partition-reduce acc with a
                        # ones matmul, reciprocal, broadcast back over
                        # partitions with a ones-row matmul
                        recip = p2.tile([1, N], F32R, tag="recip")
                        for ih in range(2):
                            l_ps = ps_s.tile([1, 1024], F32, tag="s")
                            for f in range(2):
                                nc.tensor.matmul(
                                    l_ps[:, bass.ts(f, 512)],
                                    ones_col[:],
                                    acc[:, ih * 1024 + f * 512 : ih * 1024 + (f + 1) * 512],
                                    start=True, stop=True,
                                )
                            with nc.allow_low_precision(
                                reason="f32r rounding of softmax recip "
                                       "denominators is ~2^-19 relative"):
                                nc.vector.reciprocal(
                                    recip[:, bass.ts(ih, 1024)], l_ps[:])
                        bc_sb = p2.tile([128, N], F32, tag="bcsb")
                        for ih in range(2):
                            bc_ps = ps_s.tile([128, 1024], F32, tag="s")
                            for f in range(2):
                                nc.tensor.matmul(
                                    bc_ps[:, bass.ts(f, 512)],
                                    ones_row[:],
                                    recip[0:1, ih * 1024 + f * 512 : ih * 1024 + (f + 1) * 512],
                                    start=True, stop=True,
                                )
                            nc.scalar.copy(bc_sb[:, bass.ts(ih, 1024)], bc_ps[:])
                        nc.vector.tensor_tensor(ao[:, h, :], a_ps[:], bc_sb[:],
                                                op=MULT)

                # ---------------- Phase 3: o_proj (bf16) ----------------
                with (
                    tc.tile_pool(name="p3", bufs=1) as p3,
                    tc.tile_pool(name="ps3", bufs=4, space=PSUM) as ps3,
                ):
                    wo_sb = p3.tile([128, HL, D], BF16, tag="wo")
                    nc.sync.dma_start(wo_sb[:], wo_d[:])
                    for m in range(NT):
                        st = p3.tile([128, D], F32, tag="st")
                        for f in range(4):
                            o_ps = ps3.tile([128, 512], F32, tag="o")
                            for ct in range(HL):
                                nc.tensor.matmul(
                                    o_ps[:],
                                    ao[:, ct, bass.ts(m, 128)],
                                    wo_sb[:, ct, bass.ts(f, 512)],
                                    start=(ct == 0), stop=(ct == HL - 1),
                                )
                            nc.scalar.copy(st[:, bass.ts(f, 512)], o_ps[:])
                        nc.sync.dma_start(out_d[bass.ts(m, 128), :], st[:])

    nc.compile()
    return nc


def _rope_tables():
    inv_freq = 1.0 / (10000.0 ** (np.arange(0, HD, 2, dtype=np.float32) / HD))
    pos = np.arange(N, dtype=np.float32)
    freqs = pos[:, None] * inv_freq[None, :]          # [N, HD/2]
    emb = np.concatenate([freqs, freqs], axis=-1)     # [N, HD]
    cos = np.cos(emb).astype(np.float32).T.copy()     # [HD, N]
    sin = np.sin(emb).astype(np.float32).T.copy()     # [HD, N]
    sin_signed = sin.copy()
    sin_signed[0:64] *= -1.0
    return cos, sin_signed


def _make_in_maps(x, Wq, Wk, Wv, Wo):
    cos, sin_signed = _rope_tables()
    bf = ml_dtypes.bfloat16

    in_maps = []
    for c in range(N_CORES):
        b, hg = c // 4, c % 4
        cols = slice(C * hg, C * hg + C)
        xT = np.ascontiguousarray(x[b].T)                      # [D, N]
        xt = np.ascontiguousarray(
            xT.reshape(KT, 128, NB, 512).transpose(2, 1, 0, 3)
        ).astype(bf)                                           # [NB,128,KT,512]

        def wslice(W):
            wt = W[cols, :].T                                  # [D, C]
            return np.ascontiguousarray(
                wt.reshape(KT, 128, C).transpose(1, 0, 2)
            ).astype(bf)                                       # [128, KT, C]

        wo_t = Wo[:, cols].T                                   # [C, D]
        wo = np.ascontiguousarray(
            wo_t.reshape(HL, 128, D).transpose(1, 0, 2)
        ).astype(bf)                                           # [128, HL, D]

        in_maps.append({
            "xt": xt,
            "wq": wslice(Wq),
            "wk": wslice(Wk),
            "wv": wslice(Wv),
            "wo": wo,
            "cos": cos,
            "sin": sin_signed,
            "onec": np.ones((128, 1), dtype=np.float32),
            "oner": np.ones((1, 128), dtype=np.float32),
        })
    return in_maps


def kernel(x, Wq, Wk, Wv, Wo):
    x = np.asarray(x, dtype=np.float32)
    Wq = np.asarray(Wq, dtype=np.float32)
    Wk = np.asarray(Wk, dtype=np.float32)
    Wv = np.asarray(Wv, dtype=np.float32)
    Wo = np.asarray(Wo, dtype=np.float32)

    if "nc" not in _CACHE:
        _CACHE["nc"] = _build_program()
    nc = _CACHE["nc"]

    in_maps = _make_in_maps(x, Wq, Wk, Wv, Wo)
    results = run_bass_kernel_spmd(
        nc, in_maps, core_ids=list(range(N_CORES))
    ).results

    out = np.zeros((B, N, D), dtype=np.float32)
    for c in range(N_CORES):
        out[c // 4] += results[c]["out"]
    return out



# revision 7
# speedup vs baseline: 1.2269x; 1.2269x over previous
"""MHSA + RoPE kernel for Trainium2, 8 NeuronCores.

Sharding: data-parallel over batch (B=2) x tensor-parallel over heads
(16 heads -> 4 head-groups of 4). Core c handles batch c//4, heads
[4*(c%4) : 4*(c%4)+4]. Each core computes its partial o_proj output
[N, D]; host sums the 4 partials per batch (the "all-reduce").

Per-core schedule (single TileContext scope, per-head pipeline so the
Tile scheduler can fill attention's ACT-bound PE gaps with the next
head's projection matmuls and keep the PE HAM clock gate warm):

  h=0: q0,k0 proj -> v proj (all heads) -> rope0 -> attn0
  h>0: q_h,k_h proj (overlaps attn_{h-1}) -> rope_h -> attn_h
  o_proj at the end (overlaps attn3 via the scheduler).

Everything on-chip is fp16 (same PE rate as bf16, 4x DVE modes, half
the SBUF/DMA of f32, and an 11-bit mantissa so exp values <= ~200 and
softmax denominators ~3e3 are represented to ~0.05%). PSUM stays f32.

PSUM budget (8 banks): scores [128,1024]x2 bufs = 4, PV accumulator
[128,1024]x1 = 2, shared proj/softmax-tail/o_proj pool [128,512]x2 = 2.
SBUF: the x/wq/wk/wv pool closes after the head loop so the o_proj
pool (wo + store tiles) reuses its space.
"""

import sys

sys.path.insert(0, "/opt/trn_rl_repo")

import numpy as np

import concourse.bass as bass
import concourse.tile as tile
from concourse import bacc, mybir
from concourse.bass_utils import run_bass_kernel_spmd

F32 = mybir.dt.float32
F32R = mybir.dt.float32r
F16 = mybir.dt.float16
MULT = mybir.AluOpType.mult
ADD = mybir.AluOpType.add
EXP = mybir.ActivationFunctionType.Exp
PSUM = bass.MemorySpace.PSUM

B, N, D = 2, 2048, 2048
H, HD = 16, 128
HL = 4            # local heads per core
C = HL * HD       # 512 local head cols
KT = D // 128     # 16 contraction tiles
NB = 4            # n-blocks of 512 for projections
NT = N // 128     # 16 j-tiles
SCALE = float(HD) ** -0.5
N_CORES = 8

_CACHE = {}


def _build_program():
    nc = bacc.Bacc("TRN2", target_bir_lowering=False, debug=False,
                   num_devices=N_CORES)

    xt_d = nc.dram_tensor("xt", [NB, 128, KT, 512], F16, kind="ExternalInput")
    wq_d = nc.dram_tensor("wq", [128, KT, C], F16, kind="ExternalInput")
    wk_d = nc.dram_tensor("wk", [128, KT, C], F16, kind="ExternalInput")
    wv_d = nc.dram_tensor("wv", [128, KT, C], F16, kind="ExternalInput")
    wo_d = nc.dram_tensor("wo", [128, HL, D], F16, kind="ExternalInput")
    cos_d = nc.dram_tensor("cos", [128, N], F16, kind="ExternalInput")
    sin_d = nc.dram_tensor("sin", [128, N], F16, kind="ExternalInput")
    onec_d = nc.dram_tensor("onec", [128, 1], F16, kind="ExternalInput")
    oner_d = nc.dram_tensor("oner", [1, 128], F32R, kind="ExternalInput")
    out_d = nc.dram_tensor("out", [N, D], F16, kind="ExternalOutput")

    with tile.TileContext(nc) as tc:
        with (
            tc.tile_pool(name="res", bufs=1) as res,
            tc.tile_pool(name="qk", bufs=2) as qkp,
            tc.tile_pool(name="rope", bufs=2) as ropep,
            tc.tile_pool(name="sx", bufs=3) as sxp,
            tc.tile_pool(name="accp", bufs=2) as accp,
            tc.tile_pool(name="rc", bufs=1) as rcp,
            tc.tile_pool(name="pp", bufs=2, space=PSUM) as pp,
            tc.tile_pool(name="sps", bufs=2, space=PSUM) as sps,
            tc.tile_pool(name="aps", bufs=1, space=PSUM) as aps,
        ):
            vv = res.tile([128, NT, C], F16)      # v natural [n, c]
            ao = res.tile([128, HL, N], F16)      # normalized A^T [c, n]
            cos_sb = res.tile([128, N], F16)
            sin_sb = res.tile([128, N], F16)
            onec = res.tile([128, 1], F16)
            oner = res.tile([1, 128], F32R)

            with tc.tile_pool(name="wp", bufs=1) as wp:
                x_sb = wp.tile([128, NB, KT, 512], F16, tag="x")
                wq_sb = wp.tile([128, KT, C], F16, tag="wq")
                wk_sb = wp.tile([128, KT, C], F16, tag="wk")
                wv_sb = wp.tile([128, KT, C], F16, tag="wv")

                # First-needed data in per-ktile pieces so the first
                # matmul starts after ~256KB, not after 4MB.
                for t in range(KT):
                    nc.sync.dma_start(wq_sb[:, t, :], wq_d[:, t, :])
                    nc.sync.dma_start(x_sb[:, 0, t, :], xt_d[0, :, t, :])
                for nb in range(1, NB):
                    nc.sync.dma_start(x_sb[:, nb], xt_d[nb])
                nc.sync.dma_start(wk_sb[:], wk_d[:])
                nc.sync.dma_start(wv_sb[:], wv_d[:])
                nc.sync.dma_start(cos_sb[:], cos_d[:])
                nc.sync.dma_start(sin_sb[:], sin_d[:])
                nc.sync.dma_start(onec[:], onec_d[:])
                nc.sync.dma_start(oner[:], oner_d[:])

                # Warm the ACT exp table (~2.7us) during the startup
                # DMAs so the first attention exp doesn't eat the load.
                warm = sxp.tile([128, 128], F16, tag="sx")
                nc.scalar.activation(warm[:], cos_sb[:, 0:128], EXP)

                for h in range(HL):
                    # ---- q/k projections for head h: q^T/k^T [d, n] --
                    qr = qkp.tile([128, N], F16, tag="qr")
                    kr = qkp.tile([128, N], F16, tag="kr")
                    for dst, w_sb in ((qr, wq_sb), (kr, wk_sb)):
                        for nb in range(NB):
                            ps = pp.tile([128, 512], F32, tag="pp")
                            for t in range(KT):
                                nc.tensor.matmul(
                                    ps[:],
                                    w_sb[:, t, bass.ts(h, 128)],
                                    x_sb[:, nb, t, :],
                                    start=(t == 0), stop=(t == KT - 1),
                                )
                            nc.scalar.copy(dst[:, bass.ts(nb, 512)], ps[:])

                    if h == 0:
                        # ---- v projection, all heads: v [n, c] -------
                        for m in range(NT):
                            nb, mm = m // 4, m % 4
                            ps = pp.tile([128, 512], F32, tag="pp")
                            for t in range(KT):
                                nc.tensor.matmul(
                                    ps[:],
                                    x_sb[:, nb, t, bass.ts(mm, 128)],
                                    wv_sb[:, t, :],
                                    start=(t == 0), stop=(t == KT - 1),
                                )
                            nc.scalar.copy(vv[:, m, :], ps[:])

                    # ---- RoPE in place (sin sign-folded on host) -----
                    # The d-half swap is a partition shuffle ->
                    # SBUF-SBUF DMA.
                    for src in (qr, kr):
                        tmp = ropep.tile([128, N], F16, tag="tmp")
                        nc.sync.dma_start(tmp[0:64, :], src[64:128, :])
                        nc.sync.dma_start(tmp[64:128, :], src[0:64, :])
                        nc.vector.tensor_tensor(tmp[:], tmp[:], sin_sb[:],
                                                op=MULT)
                        nc.vector.tensor_tensor(src[:], src[:], cos_sb[:],
                                                op=MULT)
                        nc.vector.tensor_tensor(src[:], src[:], tmp[:],
                                                op=ADD)

                    # ---- attention for head h ------------------------
                    for ih in range(2):
                        ihb = ih * 1024
                        a_ps = aps.tile([128, 1024], F32, tag="a")
                        acc = accp.tile([128, 1024], F16, tag="acc")
                        for j in range(NT):
                            s_ps = sps.tile([128, 1024], F32, tag="s")
                            for f in range(2):
                                nc.tensor.matmul(
                                    s_ps[:, bass.ts(f, 512)],
                                    kr[:, bass.ts(j, 128)],
                                    qr[:, ihb + f * 512:
                                        ihb + (f + 1) * 512],
                                    start=True, stop=True,
                                )
                            s_exp = sxp.tile([128, 1024], F16, tag="sx")
                            nc.scalar.activation(s_exp[:], s_ps[:], EXP,
                                                 scale=SCALE)
                            if j == 0:
                                nc.vector.tensor_copy(acc[:], s_exp[:])
                            else:
                                nc.vector.tensor_tensor(acc[:], acc[:],
                                                        s_exp[:], op=ADD)
                            for f in range(2):
                                nc.tensor.matmul(
                                    a_ps[:, bass.ts(f, 512)],
                                    vv[:, j, bass.ts(h, 128)],
                                    s_exp[:, bass.ts(f, 512)],
                                    start=(j == 0), stop=(j == NT - 1),
                                )
                        # softmax denominators: partition-reduce acc
                        # with a ones-col matmul, reciprocal, broadcast
                        # back with a ones-row matmul, normalize on DVE.
                        recip = rcp.tile([1, 1024], F32R, tag="rc")
                        for f in range(2):
                            l_ps = pp.tile([1, 512], F32, tag="pp")
                            nc.tensor.matmul(l_ps[:], onec[:],
                                             acc[:, bass.ts(f, 512)],
                                             start=True, stop=True)
                            with nc.allow_low_precision(
                                    reason="softmax recip denominators"):
                                nc.vector.reciprocal(
                                    recip[:, bass.ts(f, 512)], l_ps[:])
                        for f in range(2):
                            bc_ps = pp.tile([128, 512], F32, tag="pp")
                            nc.tensor.matmul(bc_ps[:], oner[:],
                                             recip[0:1, bass.ts(f, 512)],
                                             start=True, stop=True)
                            # DVE reads at most one PSUM operand: stage
                            # the broadcast through SBUF via ACT.
                            bc_sb = accp.tile([128, 512], F16, tag="bc")
                            nc.scalar.copy(bc_sb[:], bc_ps[:])
                            nc.vector.tensor_tensor(
                                ao[:, h, ihb + f * 512:
                                   ihb + (f + 1) * 512],
                                a_ps[:, bass.ts(f, 512)], bc_sb[:],
                                op=MULT)

            # ---- o_proj (wo/st pool reuses the closed wp space) ------
            with tc.tile_pool(name="op", bufs=1) as op:
                wo_sb = op.tile([128, HL, D], F16, tag="wo")
                nc.sync.dma_start(wo_sb[:], wo_d[:])
                for m in range(NT):
                    st = op.tile([128, D], F16, tag="st")
                    for f in range(4):
                        o_ps = pp.tile([128, 512], F32, tag="pp")
                        for ct in range(HL):
                            nc.tensor.matmul(
                                o_ps[:],
                                ao[:, ct, bass.ts(m, 128)],
                                wo_sb[:, ct, bass.ts(f, 512)],
                                start=(ct == 0), stop=(ct == HL - 1),
                            )
                        # alternate eviction engines so neither paces
                        # o_proj
                        if f % 2 == 0:
                            nc.scalar.copy(st[:, bass.ts(f, 512)],
                                           o_ps[:])
                        else:
                            nc.vector.tensor_copy(st[:, bass.ts(f, 512)],
                                                  o_ps[:])
                    nc.sync.dma_start(out_d[bass.ts(m, 128), :], st[:])

    nc.compile()
    return nc


def _rope_tables():
    inv_freq = 1.0 / (10000.0 ** (np.arange(0, HD, 2, dtype=np.float32) / HD))
    pos = np.arange(N, dtype=np.float32)
    freqs = pos[:, None] * inv_freq[None, :]          # [N, HD/2]
    emb = np.concatenate([freqs, freqs], axis=-1)     # [N, HD]
    cos = np.cos(emb).astype(np.float32).T.copy()     # [HD, N]
    sin = np.sin(emb).astype(np.float32).T.copy()     # [HD, N]
    sin_signed = sin.copy()
    sin_signed[0:64] *= -1.0
    return cos, sin_signed


def _make_in_maps(x, Wq, Wk, Wv, Wo):
    cos, sin_signed = _rope_tables()
    f16 = np.float16

    in_maps = []
    for c in range(N_CORES):
        b, hg = c // 4, c % 4
        cols = slice(C * hg, C * hg + C)
        xT = np.ascontiguousarray(x[b].T)                      # [D, N]
        xt = np.ascontiguousarray(
            xT.reshape(KT, 128, NB, 512).transpose(2, 1, 0, 3)
        ).astype(f16)                                          # [NB,128,KT,512]

        def wslice(W):
            wt = W[cols, :].T                                  # [D, C]
            return np.ascontiguousarray(
                wt.reshape(KT, 128, C).transpose(1, 0, 2)
            ).astype(f16)                                      # [128, KT, C]

        wo_t = Wo[:, cols].T                                   # [C, D]
        wo = np.ascontiguousarray(
            wo_t.reshape(HL, 128, D).transpose(1, 0, 2)
        ).astype(f16)                                          # [128, HL, D]

        in_maps.append({
            "xt": xt,
            "wq": wslice(Wq),
            "wk": wslice(Wk),
            "wv": wslice(Wv),
            "wo": wo,
            "cos": cos.astype(f16),
            "sin": sin_signed.astype(f16),
            "onec": np.ones((128, 1), dtype=f16),
            "oner": np.ones((1, 128), dtype=np.float32),
        })
    return in_maps


def kernel(x, Wq, Wk, Wv, Wo):
    x = np.asarray(x, dtype=np.float32)
    Wq = np.asarray(Wq, dtype=np.float32)
    Wv = np.asarray(Wv, dtype=np.float32)
    Wk = np.asarray(Wk, dtype=np.float32)
    Wo = np.asarray(Wo, dtype=np.float32)

    if "nc" not in _CACHE:
        _CACHE["nc"] = _build_program()
    nc = _CACHE["nc"]

    in_maps = _make_in_maps(x, Wq, Wk, Wv, Wo)
    results = run_bass_kernel_spmd(
        nc, in_maps, core_ids=list(range(N_CORES))
    ).results

    out = np.zeros((B, N, D), dtype=np.float32)
    for c in range(N_CORES):
        out[c // 4] += results[c]["out"].astype(np.float32)
    return out


# revision 8
# speedup vs baseline: 1.9673x; 1.6034x over previous
"""MHSA + RoPE kernel for Trainium2, 8 NeuronCores.

Sharding: data-parallel over batch (B=2) x tensor-parallel over heads
(16 heads -> 4 head-groups of 4). Core c handles batch c//4, heads
[4*(c%4) : 4*(c%4)+4]. Each core computes its partial o_proj output
[N, D]; host sums the 4 partials per batch (the "all-reduce").

Per-core schedule (single TileContext scope, per-head pipeline so the
Tile scheduler can fill attention's ACT-bound PE gaps with the next
head's projection matmuls and keep the PE HAM clock gate warm):

  h=0: k0,q0 proj (+inline RoPE chunks) -> v proj (all heads) -> attn0
  h>0: k_h,q_h proj + RoPE (overlaps attn_{h-1}) -> attn_h
  o_proj at the end (overlaps attn3 via the scheduler).

RoPE is applied to [128,1024] column chunks right after the projection
eviction that produces them, so the rope->scores dependency chain at a
head boundary is ~2us instead of ~10us.

Softmax denominators: an all-ones [128,128] matmul partition-reduces
acc AND broadcasts the result to all partitions in one shot; the
reciprocal runs as reciprocal_approx_fast (single DVE uop chain, ~5x
faster than reciprocal()); the normalize multiply reads a_ps (PSUM) x
bc (SBUF) on DVE.

Everything on-chip is fp16 (same PE rate as bf16, 2x DVE mode, half
the SBUF/DMA of f32, 11-bit mantissa: exp values <= ~200 and softmax
denominators ~3e3 are represented to ~0.05%). PSUM stays f32.

PSUM budget (8 banks): scores [128,1024]x2 bufs = 4, PV accumulator
[128,1024]x1 = 2, shared proj/tail/o_proj pool [128,512]x2 = 2.
o_proj additionally reuses the scores pool slots once attention ends.
"""

import sys

sys.path.insert(0, "/opt/trn_rl_repo")

import numpy as np

import concourse.bass as bass
import concourse.tile as tile
from concourse import bacc, mybir
from concourse.bass_utils import run_bass_kernel_spmd

F32 = mybir.dt.float32
F16 = mybir.dt.float16
MULT = mybir.AluOpType.mult
ADD = mybir.AluOpType.add
EXP = mybir.ActivationFunctionType.Exp
PSUM = bass.MemorySpace.PSUM

B, N, D = 2, 2048, 2048
H, HD = 16, 128
HL = 4            # local heads per core
C = HL * HD       # 512 local head cols
KT = D // 128     # 16 contraction tiles
NB = 4            # n-blocks of 512 for projections
NT = N // 128     # 16 j-tiles
SCALE = float(HD) ** -0.5
N_CORES = 8

_CACHE = {}


def _build_program():
    nc = bacc.Bacc("TRN2", target_bir_lowering=False, debug=False,
                   num_devices=N_CORES)

    xt_d = nc.dram_tensor("xt", [NB, 128, KT, 512], F16, kind="ExternalInput")
    wq_d = nc.dram_tensor("wq", [128, KT, C], F16, kind="ExternalInput")
    wk_d = nc.dram_tensor("wk", [128, KT, C], F16, kind="ExternalInput")
    wv_d = nc.dram_tensor("wv", [128, KT, C], F16, kind="ExternalInput")
    wo_d = nc.dram_tensor("wo", [128, HL, D], F16, kind="ExternalInput")
    cos_d = nc.dram_tensor("cos", [128, N], F16, kind="ExternalInput")
    sin_d = nc.dram_tensor("sin", [128, N], F16, kind="ExternalInput")
    onem_d = nc.dram_tensor("onem", [128, 128], F16, kind="ExternalInput")
    out_d = nc.dram_tensor("out", [N, D], F16, kind="ExternalOutput")

    with tile.TileContext(nc) as tc:
        with (
            tc.tile_pool(name="res", bufs=1) as res,
            tc.tile_pool(name="qk", bufs=2) as qkp,
            tc.tile_pool(name="rope", bufs=2) as ropep,
            tc.tile_pool(name="sx", bufs=3) as sxp,
            tc.tile_pool(name="accp", bufs=2) as accp,
            tc.tile_pool(name="pp", bufs=2, space=PSUM) as pp,
            tc.tile_pool(name="sps", bufs=2, space=PSUM) as sps,
            tc.tile_pool(name="aps", bufs=1, space=PSUM) as aps,
        ):
            vv = res.tile([128, NT, C], F16)      # v natural [n, c]
            ao = res.tile([128, HL, N], F16)      # normalized A^T [c, n]
            cos_sb = res.tile([128, N], F16)
            sin_sb = res.tile([128, N], F16)
            onem = res.tile([128, 128], F16)

            def rope_chunk(dst, lo):
                # in-place RoPE on dst[:, lo:lo+1024]; sin sign-folded
                # on host. The d-half swap is a partition shuffle ->
                # SBUF-SBUF DMA.
                sl = slice(lo, lo + 1024)
                tmp = ropep.tile([128, 1024], F16, tag="tmp")
                nc.sync.dma_start(tmp[0:64, :], dst[64:128, sl])
                nc.sync.dma_start(tmp[64:128, :], dst[0:64, sl])
                nc.vector.tensor_tensor(tmp[:], tmp[:], sin_sb[:, sl],
                                        op=MULT)
                nc.vector.tensor_tensor(dst[:, sl], dst[:, sl],
                                        cos_sb[:, sl], op=MULT)
                nc.vector.tensor_tensor(dst[:, sl], dst[:, sl], tmp[:],
                                        op=ADD)

            with tc.tile_pool(name="wp", bufs=1) as wp:
                x_sb = wp.tile([128, NB, KT, 512], F16, tag="x")
                wq_sb = wp.tile([128, KT, C], F16, tag="wq")
                wk_sb = wp.tile([128, KT, C], F16, tag="wk")
                wv_sb = wp.tile([128, KT, C], F16, tag="wv")

                # First-needed data first, in per-ktile pieces so the
                # first matmuls start after ~256KB, not after 4MB.
                for t in range(KT):
                    nc.sync.dma_start(wk_sb[:, t, :], wk_d[:, t, :])
                    nc.sync.dma_start(x_sb[:, 0, t, :], xt_d[0, :, t, :])
                nc.sync.dma_start(x_sb[:, 1], xt_d[1])
                nc.sync.dma_start(wq_sb[:], wq_d[:])
                nc.sync.dma_start(x_sb[:, 2], xt_d[2])
                nc.sync.dma_start(x_sb[:, 3], xt_d[3])
                nc.sync.dma_start(wv_sb[:], wv_d[:])
                nc.sync.dma_start(cos_sb[:], cos_d[:])
                nc.sync.dma_start(sin_sb[:], sin_d[:])
                nc.sync.dma_start(onem[:], onem_d[:])

                # Warm the ACT exp table (~2.7us) during the startup
                # DMAs so the first attention exp doesn't eat the load.
                warm = sxp.tile([128, 128], F16, tag="sx")
                nc.scalar.activation(warm[:], cos_sb[:, 0:128], EXP)

                for h in range(HL):
                    # ---- k/q projections for head h: k^T/q^T [d, n],
                    # RoPE chunks inline after the evictions that
                    # complete each 1024-column half. k first so scores
                    # j-tiles unblock as early as possible.
                    qr = qkp.tile([128, N], F16, tag="qr")
                    kr = qkp.tile([128, N], F16, tag="kr")
                    for dst, w_sb in ((kr, wk_sb), (qr, wq_sb)):
                        for nb in range(NB):
                            ps = pp.tile([128, 512], F32, tag="pp")
                            for t in range(KT):
                                nc.tensor.matmul(
                                    ps[:],
                                    w_sb[:, t, bass.ts(h, 128)],
                                    x_sb[:, nb, t, :],
                                    start=(t == 0), stop=(t == KT - 1),
                                )
                            nc.scalar.copy(dst[:, bass.ts(nb, 512)], ps[:])
                            if nb % 2 == 1:
                                rope_chunk(dst, (nb - 1) * 512)

                    if h == 0:
                        # ---- v projection, all heads: v [n, c] -------
                        for m in range(NT):
                            nb, mm = m // 4, m % 4
                            ps = pp.tile([128, 512], F32, tag="pp")
                            for t in range(KT):
                                nc.tensor.matmul(
                                    ps[:],
                                    x_sb[:, nb, t, bass.ts(mm, 128)],
                                    wv_sb[:, t, :],
                                    start=(t == 0), stop=(t == KT - 1),
                                )
                            nc.scalar.copy(vv[:, m, :], ps[:])

                    # ---- attention for head h ------------------------
                    for ih in range(2):
                        ihb = ih * 1024
                        a_ps = aps.tile([128, 1024], F32, tag="a")
                        acc = accp.tile([128, 1024], F16, tag="acc")
                        for j in range(NT):
                            s_ps = sps.tile([128, 1024], F32, tag="s")
                            for f in range(2):
                                nc.tensor.matmul(
                                    s_ps[:, bass.ts(f, 512)],
                                    kr[:, bass.ts(j, 128)],
                                    qr[:, ihb + f * 512:
                                        ihb + (f + 1) * 512],
                                    start=True, stop=True,
                                )
                            s_exp = sxp.tile([128, 1024], F16, tag="sx")
                            nc.scalar.activation(s_exp[:], s_ps[:], EXP,
                                                 scale=SCALE)
                            if j == 0:
                                nc.vector.tensor_copy(acc[:], s_exp[:])
                            else:
                                nc.vector.tensor_tensor(acc[:], acc[:],
                                                        s_exp[:], op=ADD)
                            for f in range(2):
                                nc.tensor.matmul(
                                    a_ps[:, bass.ts(f, 512)],
                                    vv[:, j, bass.ts(h, 128)],
                                    s_exp[:, bass.ts(f, 512)],
                                    start=(j == 0), stop=(j == NT - 1),
                                )
                        # softmax denominators: the all-ones [128,128]
                        # matmul partition-reduces acc AND broadcasts
                        # den[i] to every partition; fast approx
                        # reciprocal; normalize on DVE.
                        for f in range(2):
                            den_ps = pp.tile([128, 512], F32, tag="pp")
                            nc.tensor.matmul(den_ps[:], onem[:],
                                             acc[:, bass.ts(f, 512)],
                                             start=True, stop=True)
                            bc_sb = accp.tile([128, 512], F32, tag="bc")
                            with nc.allow_low_precision(
                                    reason="softmax denominators: approx "
                                           "recip is ~51 ULP"):
                                nc.vector.reciprocal_approx_fast(
                                    out=bc_sb[:], in_=den_ps[:])
                            nc.vector.tensor_tensor(
                                ao[:, h, ihb + f * 512:
                                   ihb + (f + 1) * 512],
                                a_ps[:, bass.ts(f, 512)], bc_sb[:],
                                op=MULT)

            # ---- o_proj (wo/st pools reuse the closed wp space) ------
            with (
                tc.tile_pool(name="op", bufs=1) as op,
                tc.tile_pool(name="stp", bufs=3) as stp,
            ):
                wo_sb = op.tile([128, HL, D], F16, tag="wo")
                nc.sync.dma_start(wo_sb[:], wo_d[:])
                for m in range(NT):
                    st = stp.tile([128, D], F16, tag="st")
                    for f in range(4):
                        # spread o_ps over the pp pool and the (now
                        # idle) scores pool for 4-deep PSUM pipelining
                        if f % 2 == 0:
                            o_ps = pp.tile([128, 512], F32, tag="pp")
                        else:
                            o_ps = sps.tile([128, 512], F32, tag="s")
                        for ct in range(HL):
                            nc.tensor.matmul(
                                o_ps[:],
                                ao[:, ct, bass.ts(m, 128)],
                                wo_sb[:, ct, bass.ts(f, 512)],
                                start=(ct == 0), stop=(ct == HL - 1),
                            )
                        # alternate eviction engines so neither paces
                        # o_proj
                        if f % 2 == 0:
                            nc.scalar.copy(st[:, bass.ts(f, 512)],
                                           o_ps[:])
                        else:
                            nc.vector.tensor_copy(st[:, bass.ts(f, 512)],
                                                  o_ps[:])
                    nc.sync.dma_start(out_d[bass.ts(m, 128), :], st[:])

    nc.compile()
    return nc


def _rope_tables():
    inv_freq = 1.0 / (10000.0 ** (np.arange(0, HD, 2, dtype=np.float32) / HD))
    pos = np.arange(N, dtype=np.float32)
    freqs = pos[:, None] * inv_freq[None, :]          # [N, HD/2]
    emb = np.concatenate([freqs, freqs], axis=-1)     # [N, HD]
    cos = np.cos(emb).astype(np.float32).T.copy()     # [HD, N]
    sin = np.sin(emb).astype(np.float32).T.copy()     # [HD, N]
    sin_signed = sin.copy()
    sin_signed[0:64] *= -1.0
    return cos, sin_signed


def _make_in_maps(x, Wq, Wk, Wv, Wo):
    cos, sin_signed = _rope_tables()
    f16 = np.float16

    in_maps = []
    for c in range(N_CORES):
        b, hg = c // 4, c % 4
        cols = slice(C * hg, C * hg + C)
        xT = np.ascontiguousarray(x[b].T)                      # [D, N]
        xt = np.ascontiguousarray(
            xT.reshape(KT, 128, NB, 512).transpose(2, 1, 0, 3)
        ).astype(f16)                                          # [NB,128,KT,512]

        def wslice(W):
            wt = W[cols, :].T                                  # [D, C]
            return np.ascontiguousarray(
                wt.reshape(KT, 128, C).transpose(1, 0, 2)
            ).astype(f16)                                      # [128, KT, C]

        wo_t = Wo[:, cols].T                                   # [C, D]
        wo = np.ascontiguousarray(
            wo_t.reshape(HL, 128, D).transpose(1, 0, 2)
        ).astype(f16)                                          # [128, HL, D]

        in_maps.append({
            "xt": xt,
            "wq": wslice(Wq),
            "wk": wslice(Wk),
            "wv": wslice(Wv),
            "wo": wo,
            "cos": cos.astype(f16),
            "sin": sin_signed.astype(f16),
            "onem": np.ones((128, 128), dtype=f16),
        })
    return in_maps


def kernel(x, Wq, Wk, Wv, Wo):
    x = np.asarray(x, dtype=np.float32)
    Wq = np.asarray(Wq, dtype=np.float32)
    Wk = np.asarray(Wk, dtype=np.float32)
    Wv = np.asarray(Wv, dtype=np.float32)
    Wo = np.asarray(Wo, dtype=np.float32)

    if "nc" not in _CACHE:
        _CACHE["nc"] = _build_program()
    nc = _CACHE["nc"]

    in_maps = _make_in_maps(x, Wq, Wk, Wv, Wo)
    results = run_bass_kernel_spmd(
        nc, in_maps, core_ids=list(range(N_CORES))
    ).results

    out = np.zeros((B, N, D), dtype=np.float32)
    for c in range(N_CORES):
        out[c // 4] += results[c]["out"].astype(np.float32)
    return out


# revision 13
# speedup vs baseline: 1.9803x; 1.0066x over previous
"""MHSA + RoPE kernel for Trainium2, 8 NeuronCores.

Sharding: data-parallel over batch (B=2) x tensor-parallel over heads
(16 heads -> 4 head-groups of 4). Core c handles batch c//4, heads
[4*(c%4) : 4*(c%4)+4]. Each core computes its partial o_proj output
[N, D]; host sums the 4 partials per batch (the "all-reduce").

Per-core schedule (single TileContext scope, per-head pipeline so the
Tile scheduler can fill attention's ACT-bound PE gaps with the next
head's projection matmuls and keep the PE HAM clock gate warm):

  h=0: k0,q0 proj (+inline RoPE chunks) -> v proj (all heads) -> attn0
  h>0: k_h,q_h proj + RoPE (overlaps attn_{h-1}) -> attn_h
  o_proj at the end (overlaps attn3 via the scheduler).

RoPE is applied to [128,1024] column chunks right after the projection
eviction that produces them, so the rope->scores dependency chain at a
head boundary is ~2us instead of ~10us.

Softmax denominators: an all-ones [128,128] matmul partition-reduces
acc AND broadcasts the result to all partitions in one shot; the
reciprocal runs as reciprocal_approx_fast (single DVE uop chain, ~5x
faster than reciprocal()); the normalize multiply reads a_ps (PSUM) x
bc (SBUF) on DVE.

Everything on-chip is fp16 (same PE rate as bf16, 2x DVE mode, half
the SBUF/DMA of f32, 11-bit mantissa: exp values <= ~200 and softmax
denominators ~3e3 are represented to ~0.05%). PSUM stays f32.

PSUM budget (8 banks): scores [128,1024]x2 bufs = 4, PV accumulator
[128,1024]x1 = 2, shared proj/tail/o_proj pool [128,512]x2 = 2.
o_proj additionally reuses the scores pool slots once attention ends.
"""

import sys

sys.path.insert(0, "/opt/trn_rl_repo")

import numpy as np

import concourse.bass as bass
import concourse.tile as tile
from concourse import bacc, mybir
from concourse.bass_utils import run_bass_kernel_spmd

F32 = mybir.dt.float32
F16 = mybir.dt.float16
MULT = mybir.AluOpType.mult
ADD = mybir.AluOpType.add
EXP = mybir.ActivationFunctionType.Exp
PSUM = bass.MemorySpace.PSUM

B, N, D = 2, 2048, 2048
H, HD = 16, 128
HL = 4            # local heads per core
C = HL * HD       # 512 local head cols
KT = D // 128     # 16 contraction tiles
NB = 4            # n-blocks of 512 for projections
NT = N // 128     # 16 j-tiles
SCALE = float(HD) ** -0.5
N_CORES = 8

_CACHE = {}


def _build_program():
    nc = bacc.Bacc("TRN2", target_bir_lowering=False, debug=False,
                   num_devices=N_CORES)

    xt_d = nc.dram_tensor("xt", [NB, 128, KT, 512], F16, kind="ExternalInput")
    wq_d = nc.dram_tensor("wq", [128, KT, C], F16, kind="ExternalInput")
    wk_d = nc.dram_tensor("wk", [128, KT, C], F16, kind="ExternalInput")
    wv_d = nc.dram_tensor("wv", [128, KT, C], F16, kind="ExternalInput")
    wo_d = nc.dram_tensor("wo", [128, HL, D], F16, kind="ExternalInput")
    cos_d = nc.dram_tensor("cos", [128, N], F16, kind="ExternalInput")
    sin_d = nc.dram_tensor("sin", [128, N], F16, kind="ExternalInput")
    onem_d = nc.dram_tensor("onem", [128, 128], F16, kind="ExternalInput")
    out_d = nc.dram_tensor("out", [N, D], F16, kind="ExternalOutput")

    with tile.TileContext(nc) as tc:
        with (
            tc.tile_pool(name="res", bufs=1) as res,
            tc.tile_pool(name="qk", bufs=2) as qkp,
            tc.tile_pool(name="rope", bufs=2) as ropep,
            tc.tile_pool(name="sx", bufs=4) as sxp,
            tc.tile_pool(name="accp", bufs=2) as accp,
            tc.tile_pool(name="pp", bufs=2, space=PSUM) as pp,
            tc.tile_pool(name="sps", bufs=2, space=PSUM) as sps,
            tc.tile_pool(name="aps", bufs=2, space=PSUM) as aps,
        ):
            vv = res.tile([128, NT, C], F16)      # v natural [n, c]
            ao = res.tile([128, HL, N], F16)      # normalized A^T [c, n]
            cos_sb = res.tile([128, N], F16)
            sin_sb = res.tile([128, N], F16)
            onem = res.tile([128, 128], F16)

            def rope_chunk(dst, lo):
                # in-place RoPE on dst[:, lo:lo+1024]; sin sign-folded
                # on host. The d-half swap is a partition shuffle ->
                # SBUF-SBUF DMA.
                sl = slice(lo, lo + 1024)
                tmp = ropep.tile([128, 1024], F16, tag="tmp")
                nc.sync.dma_start(tmp[0:64, :], dst[64:128, sl])
                nc.sync.dma_start(tmp[64:128, :], dst[0:64, sl])
                nc.vector.tensor_tensor(tmp[:], tmp[:], sin_sb[:, sl],
                                        op=MULT)
                nc.vector.tensor_tensor(dst[:, sl], dst[:, sl],
                                        cos_sb[:, sl], op=MULT)
                nc.vector.tensor_tensor(dst[:, sl], dst[:, sl], tmp[:],
                                        op=ADD)

            with tc.tile_pool(name="wp", bufs=1) as wp:
                x_sb = wp.tile([128, NB, KT, 512], F16, tag="x")
                wq_sb = wp.tile([128, KT, C], F16, tag="wq")
                wk_sb = wp.tile([128, KT, C], F16, tag="wk")
                wv_sb = wp.tile([128, KT, C], F16, tag="wv")

                # DMA order matches consumption order: k-proj eats
                # wk+x0..x3, RoPE needs cos/sin right after x1 is
                # consumed, then q-proj (wq) and v-proj (wv).
                # Per-ktile pieces so the first matmuls start after
                # ~256KB, not after 4MB.
                for t in range(KT):
                    nc.sync.dma_start(wk_sb[:, t, :], wk_d[:, t, :])
                    nc.sync.dma_start(x_sb[:, 0, t, :], xt_d[0, :, t, :])
                for t in range(KT):
                    nc.sync.dma_start(x_sb[:, 1, t, :], xt_d[1, :, t, :])
                nc.sync.dma_start(cos_sb[:], cos_d[:])
                nc.sync.dma_start(sin_sb[:], sin_d[:])
                nc.sync.dma_start(onem[:], onem_d[:])
                nc.sync.dma_start(x_sb[:, 2], xt_d[2])
                nc.sync.dma_start(x_sb[:, 3], xt_d[3])
                nc.sync.dma_start(wq_sb[:], wq_d[:])
                nc.sync.dma_start(wv_sb[:], wv_d[:])

                # Warm the ACT exp table (~2.7us) during the startup
                # DMAs so the first attention exp doesn't eat the load.
                warm = sxp.tile([128, 128], F16, tag="sx")
                nc.scalar.activation(warm[:], cos_sb[:, 0:128], EXP)

                for h in range(HL):
                    # ---- k/q projections for head h: k^T/q^T [d, n],
                    # RoPE chunks inline after the evictions that
                    # complete each 1024-column half. k first so scores
                    # j-tiles unblock as early as possible.
                    qr = qkp.tile([128, N], F16, tag="qr")
                    kr = qkp.tile([128, N], F16, tag="kr")
                    for dst, w_sb in ((kr, wk_sb), (qr, wq_sb)):
                        for nb in range(NB):
                            ps = pp.tile([128, 512], F32, tag="pp")
                            for t in range(KT):
                                nc.tensor.matmul(
                                    ps[:],
                                    w_sb[:, t, bass.ts(h, 128)],
                                    x_sb[:, nb, t, :],
                                    start=(t == 0), stop=(t == KT - 1),
                                )
                            nc.scalar.copy(dst[:, bass.ts(nb, 512)], ps[:])
                            if nb % 2 == 1:
                                rope_chunk(dst, (nb - 1) * 512)

                    if h == 0:
                        # ---- v projection, all heads: v [n, c] -------
                        for m in range(NT):
                            nb, mm = m // 4, m % 4
                            ps = pp.tile([128, 512], F32, tag="pp")
                            for t in range(KT):
                                nc.tensor.matmul(
                                    ps[:],
                                    x_sb[:, nb, t, bass.ts(mm, 128)],
                                    wv_sb[:, t, :],
                                    start=(t == 0), stop=(t == KT - 1),
                                )
                            nc.scalar.copy(vv[:, m, :], ps[:])

                    # ---- attention for head h ------------------------
                    for ih in range(2):
                        ihb = ih * 1024
                        # per-512-col accumulator tiles (1 bank each,
                        # 2 bufs): normalize of f=0 can release its
                        # bank while f=1 still accumulates, halving the
                        # ih-boundary WAR stall.
                        a_ps0 = aps.tile([128, 512], F32, tag="a")
                        a_ps1 = aps.tile([128, 512], F32, tag="a")
                        a_ps = (a_ps0, a_ps1)
                        acc = accp.tile([128, 1024], F16, tag="acc")
                        for j in range(NT):
                            s_ps = sps.tile([128, 1024], F32, tag="s")
                            for f in range(2):
                                nc.tensor.matmul(
                                    s_ps[:, bass.ts(f, 512)],
                                    kr[:, bass.ts(j, 128)],
                                    qr[:, ihb + f * 512:
                                        ihb + (f + 1) * 512],
                                    start=True, stop=True,
                                )
                            s_exp = sxp.tile([128, 1024], F16, tag="sx")
                            nc.scalar.activation(s_exp[:], s_ps[:], EXP,
                                                 scale=SCALE)
                            if j == 0:
                                nc.vector.tensor_copy(acc[:], s_exp[:])
                            else:
                                nc.vector.tensor_tensor(acc[:], acc[:],
                                                        s_exp[:], op=ADD)
                            for f in range(2):
                                nc.tensor.matmul(
                                    a_ps[f][:],
                                    vv[:, j, bass.ts(h, 128)],
                                    s_exp[:, bass.ts(f, 512)],
                                    start=(j == 0), stop=(j == NT - 1),
                                )
                        # softmax denominators: the all-ones [128,128]
                        # matmul partition-reduces acc AND broadcasts
                        # den[i] to every partition; fast approx
                        # reciprocal; normalize on DVE.
                        for f in range(2):
                            den_ps = pp.tile([128, 512], F32, tag="pp")
                            nc.tensor.matmul(den_ps[:], onem[:],
                                             acc[:, bass.ts(f, 512)],
                                             start=True, stop=True)
                            bc_sb = accp.tile([128, 512], F32, tag="bc")
                            with nc.allow_low_precision(
                                    reason="softmax denominators: approx "
                                           "recip is ~51 ULP"):
                                nc.vector.reciprocal_approx_fast(
                                    out=bc_sb[:], in_=den_ps[:])
                            nc.vector.tensor_tensor(
                                ao[:, h, ihb + f * 512:
                                   ihb + (f + 1) * 512],
                                a_ps[f][:], bc_sb[:],
                                op=MULT)

            # ---- o_proj (wo/st pools reuse the closed wp space) ------
            with (
                tc.tile_pool(name="op", bufs=1) as op,
                tc.tile_pool(name="stp", bufs=3) as stp,
            ):
                wo_sb = op.tile([128, HL, D], F16, tag="wo")
                nc.sync.dma_start(wo_sb[:], wo_d[:])
                for m in range(NT):
                    st = stp.tile([128, D], F16, tag="st")
                    for f in range(4):
                        # spread o_ps over the pp pool and the (now
                        # idle) scores pool for 4-deep PSUM pipelining
                        if f % 2 == 0:
                            o_ps = pp.tile([128, 512], F32, tag="pp")
                        else:
                            o_ps = sps.tile([128, 512], F32, tag="s")
                        for ct in range(HL):
                            nc.tensor.matmul(
                                o_ps[:],
                                ao[:, ct, bass.ts(m, 128)],
                                wo_sb[:, ct, bass.ts(f, 512)],
                                start=(ct == 0), stop=(ct == HL - 1),
                            )
                        # alternate eviction engines so neither paces
                        # o_proj
                        if f % 2 == 0:
                            nc.scalar.copy(st[:, bass.ts(f, 512)],
                                           o_ps[:])
                        else:
                            nc.vector.tensor_copy(st[:, bass.ts(f, 512)],
                                                  o_ps[:])
                    nc.sync.dma_start(out_d[bass.ts(m, 128), :], st[:])

    nc.compile()
    return nc


def _rope_tables():
    inv_freq = 1.0 / (10000.0 ** (np.arange(0, HD, 2, dtype=np.float32) / HD))
    pos = np.arange(N, dtype=np.float32)
    freqs = pos[:, None] * inv_freq[None, :]          # [N, HD/2]
    emb = np.concatenate([freqs, freqs], axis=-1)     # [N, HD]
    cos = np.cos(emb).astype(np.float32).T.copy()     # [HD, N]
    sin = np.sin(emb).astype(np.float32).T.copy()     # [HD, N]
    sin_signed = sin.copy()
    sin_signed[0:64] *= -1.0
    return cos, sin_signed


def _make_in_maps(x, Wq, Wk, Wv, Wo):
    cos, sin_signed = _rope_tables()
    f16 = np.float16

    in_maps = []
    for c in range(N_CORES):
        b, hg = c // 4, c % 4
        cols = slice(C * hg, C * hg + C)
        xT = np.ascontiguousarray(x[b].T)                      # [D, N]
        xt = np.ascontiguousarray(
            xT.reshape(KT, 128, NB, 512).transpose(2, 1, 0, 3)
        ).astype(f16)                                          # [NB,128,KT,512]

        def wslice(W):
            wt = W[cols, :].T                                  # [D, C]
            return np.ascontiguousarray(
                wt.reshape(KT, 128, C).transpose(1, 0, 2)
            ).astype(f16)                                      # [128, KT, C]

        wo_t = Wo[:, cols].T                                   # [C, D]
        wo = np.ascontiguousarray(
            wo_t.reshape(HL, 128, D).transpose(1, 0, 2)
        ).astype(f16)                                          # [128, HL, D]

        in_maps.append({
            "xt": xt,
            "wq": wslice(Wq),
            "wk": wslice(Wk),
            "wv": wslice(Wv),
            "wo": wo,
            "cos": cos.astype(f16),
            "sin": sin_signed.astype(f16),
            "onem": np.ones((128, 128), dtype=f16),
        })
    return in_maps


def kernel(x, Wq, Wk, Wv, Wo):
    x = np.asarray(x, dtype=np.float32)
    Wq = np.asarray(Wq, dtype=np.float32)
    Wk = np.asarray(Wk, dtype=np.float32)
    Wv = np.asarray(Wv, dtype=np.float32)
    Wo = np.asarray(Wo, dtype=np.float32)

    if "nc" not in _CACHE:
        _CACHE["nc"] = _build_program()
    nc = _CACHE["nc"]

    in_maps = _make_in_maps(x, Wq, Wk, Wv, Wo)
    results = run_bass_kernel_spmd(
        nc, in_maps, core_ids=list(range(N_CORES))
    ).results

    out = np.zeros((B, N, D), dtype=np.float32)
    for c in range(N_CORES):
        out[c // 4] += results[c]["out"].astype(np.float32)
    return out


# revision 16
# speedup vs baseline: 1.9871x; 1.0034x over previous
"""MHSA + RoPE kernel for Trainium2, 8 NeuronCores.

Sharding: data-parallel over batch (B=2) x tensor-parallel over heads
(16 heads -> 4 head-groups of 4). Core c handles batch c//4, heads
[4*(c%4) : 4*(c%4)+4]. Each core computes its partial o_proj output
[N, D]; host sums the 4 partials per batch (the "all-reduce").

Per-core schedule (single TileContext scope, per-head pipeline so the
Tile scheduler can fill attention's ACT-bound PE gaps with the next
head's projection matmuls and keep the PE HAM clock gate warm):

  h=0: k0,q0 proj (+inline RoPE chunks) -> v proj (all heads) -> attn0
  h>0: k_h,q_h proj + RoPE (overlaps attn_{h-1}) -> attn_h
  o_proj at the end (overlaps attn3 via the scheduler).

RoPE is applied to [128,1024] column chunks right after the projection
eviction that produces them, so the rope->scores dependency chain at a
head boundary is ~2us instead of ~10us.

Softmax denominators: an all-ones [128,128] matmul partition-reduces
acc AND broadcasts the result to all partitions in one shot; the
reciprocal runs as reciprocal_approx_fast (single DVE uop chain, ~5x
faster than reciprocal()); the normalize multiply reads a_ps (PSUM) x
bc (SBUF) on DVE.

Everything on-chip is fp16 (same PE rate as bf16, 2x DVE mode, half
the SBUF/DMA of f32, 11-bit mantissa: exp values <= ~200 and softmax
denominators ~3e3 are represented to ~0.05%). PSUM stays f32.

PSUM budget (8 banks): scores [128,1024]x2 bufs = 4, PV accumulator
[128,1024]x1 = 2, shared proj/tail/o_proj pool [128,512]x2 = 2.
o_proj additionally reuses the scores pool slots once attention ends.
"""

import sys

sys.path.insert(0, "/opt/trn_rl_repo")

import numpy as np

import concourse.bass as bass
import concourse.tile as tile
from concourse import bacc, mybir
from concourse.bass_utils import run_bass_kernel_spmd

F32 = mybir.dt.float32
F16 = mybir.dt.float16
MULT = mybir.AluOpType.mult
ADD = mybir.AluOpType.add
EXP = mybir.ActivationFunctionType.Exp
PSUM = bass.MemorySpace.PSUM

B, N, D = 2, 2048, 2048
H, HD = 16, 128
HL = 4            # local heads per core
C = HL * HD       # 512 local head cols
KT = D // 128     # 16 contraction tiles
NB = 4            # n-blocks of 512 for projections
NT = N // 128     # 16 j-tiles
SCALE = float(HD) ** -0.5
N_CORES = 8

_CACHE = {}


def _build_program():
    nc = bacc.Bacc("TRN2", target_bir_lowering=False, debug=False,
                   num_devices=N_CORES)

    xt_d = nc.dram_tensor("xt", [NB, 128, KT, 512], F16, kind="ExternalInput")
    wq_d = nc.dram_tensor("wq", [128, KT, C], F16, kind="ExternalInput")
    wk_d = nc.dram_tensor("wk", [128, KT, C], F16, kind="ExternalInput")
    wv_d = nc.dram_tensor("wv", [128, KT, C], F16, kind="ExternalInput")
    wo_d = nc.dram_tensor("wo", [128, HL, D], F16, kind="ExternalInput")
    cos_d = nc.dram_tensor("cos", [128, N], F16, kind="ExternalInput")
    sin_d = nc.dram_tensor("sin", [128, N], F16, kind="ExternalInput")
    onem_d = nc.dram_tensor("onem", [128, 128], F16, kind="ExternalInput")
    out_d = nc.dram_tensor("out", [N, D], F16, kind="ExternalOutput")

    with tile.TileContext(nc) as tc:
        with (
            tc.tile_pool(name="res", bufs=1) as res,
            tc.tile_pool(name="qk", bufs=2) as qkp,
            tc.tile_pool(name="rope", bufs=2) as ropep,
            tc.tile_pool(name="sx", bufs=4) as sxp,
            tc.tile_pool(name="accp", bufs=2) as accp,
            tc.tile_pool(name="pp", bufs=2, space=PSUM) as pp,
            tc.tile_pool(name="sps", bufs=2, space=PSUM) as sps,
            tc.tile_pool(name="aps", bufs=2, space=PSUM) as aps,
        ):
            vv = res.tile([128, NT, C], F16)      # v natural [n, c]
            ao = res.tile([128, HL, N], F16)      # normalized A^T [c, n]
            cos_sb = res.tile([128, N], F16)
            sin_sb = res.tile([128, N], F16)
            onem = res.tile([128, 128], F16)

            def rope_chunk(dst, lo):
                # in-place RoPE on dst[:, lo:lo+1024]; sin sign-folded
                # on host. The d-half swap is a partition shuffle ->
                # SBUF-SBUF DMA.
                sl = slice(lo, lo + 1024)
                tmp = ropep.tile([128, 1024], F16, tag="tmp")
                nc.sync.dma_start(tmp[0:64, :], dst[64:128, sl])
                nc.sync.dma_start(tmp[64:128, :], dst[0:64, sl])
                nc.vector.tensor_tensor(tmp[:], tmp[:], sin_sb[:, sl],
                                        op=MULT)
                nc.vector.tensor_tensor(dst[:, sl], dst[:, sl],
                                        cos_sb[:, sl], op=MULT)
                nc.vector.tensor_tensor(dst[:, sl], dst[:, sl], tmp[:],
                                        op=ADD)

            with tc.tile_pool(name="wp", bufs=1) as wp:
                x_sb = wp.tile([128, NB, KT, 512], F16, tag="x")
                wq_sb = wp.tile([128, KT, C], F16, tag="wq")
                wk_sb = wp.tile([128, KT, C], F16, tag="wk")
                wv_sb = wp.tile([128, KT, C], F16, tag="wv")

                # DMA order matches consumption order: k-proj eats
                # wk+x0..x3, RoPE needs cos/sin right after x1 is
                # consumed, then q-proj (wq) and v-proj (wv).
                # Per-ktile pieces so the first matmuls start after
                # ~256KB, not after 4MB.
                for t in range(KT):
                    nc.sync.dma_start(wk_sb[:, t, :], wk_d[:, t, :])
                    nc.sync.dma_start(x_sb[:, 0, t, :], xt_d[0, :, t, :])
                for t in range(KT):
                    nc.sync.dma_start(x_sb[:, 1, t, :], xt_d[1, :, t, :])
                nc.sync.dma_start(cos_sb[:], cos_d[:])
                nc.sync.dma_start(sin_sb[:], sin_d[:])
                nc.sync.dma_start(onem[:], onem_d[:])
                for t in range(KT):
                    nc.sync.dma_start(wq_sb[:, t, :], wq_d[:, t, :])
                nc.sync.dma_start(x_sb[:, 2], xt_d[2])
                nc.sync.dma_start(x_sb[:, 3], xt_d[3])
                nc.sync.dma_start(wv_sb[:], wv_d[:])

                # Warm the ACT exp table (~2.7us) during the startup
                # DMAs so the first attention exp doesn't eat the load.
                warm = sxp.tile([128, 128], F16, tag="sx")
                nc.scalar.activation(warm[:], cos_sb[:, 0:128], EXP)

                for h in range(HL):
                    # ---- k/q projections for head h: k^T/q^T [d, n],
                    # RoPE chunks inline after the evictions that
                    # complete each 1024-column half. k first so scores
                    # j-tiles unblock as early as possible.
                    qr = qkp.tile([128, N], F16, tag="qr")
                    kr = qkp.tile([128, N], F16, tag="kr")
                    if h == 0:
                        # startup: nb-pair order matches DMA arrival
                        # (wk+x0, x1, wq, x2, x3) so the PE is never
                        # waiting on a transfer it doesn't need yet
                        order = [(kr, wk_sb, 0), (kr, wk_sb, 1),
                                 (qr, wq_sb, 0), (qr, wq_sb, 1),
                                 (kr, wk_sb, 2), (kr, wk_sb, 3),
                                 (qr, wq_sb, 2), (qr, wq_sb, 3)]
                    else:
                        order = [(kr, wk_sb, nb) for nb in range(NB)] + \
                                [(qr, wq_sb, nb) for nb in range(NB)]
                    for dst, w_sb, nb in order:
                        ps = pp.tile([128, 512], F32, tag="pp")
                        for t in range(KT):
                            nc.tensor.matmul(
                                ps[:],
                                w_sb[:, t, bass.ts(h, 128)],
                                x_sb[:, nb, t, :],
                                start=(t == 0), stop=(t == KT - 1),
                            )
                        nc.scalar.copy(dst[:, bass.ts(nb, 512)], ps[:])
                        if nb % 2 == 1:
                            rope_chunk(dst, (nb - 1) * 512)

                    if h == 0:
                        # ---- v projection, all heads: v [n, c] -------
                        for m in range(NT):
                            nb, mm = m // 4, m % 4
                            ps = pp.tile([128, 512], F32, tag="pp")
                            for t in range(KT):
                                nc.tensor.matmul(
                                    ps[:],
                                    x_sb[:, nb, t, bass.ts(mm, 128)],
                                    wv_sb[:, t, :],
                                    start=(t == 0), stop=(t == KT - 1),
                                )
                            nc.scalar.copy(vv[:, m, :], ps[:])

                    # ---- attention for head h ------------------------
                    for ih in range(2):
                        ihb = ih * 1024
                        # per-512-col accumulator tiles (1 bank each,
                        # 2 bufs): normalize of f=0 can release its
                        # bank while f=1 still accumulates, halving the
                        # ih-boundary WAR stall.
                        a_ps0 = aps.tile([128, 512], F32, tag="a")
                        a_ps1 = aps.tile([128, 512], F32, tag="a")
                        a_ps = (a_ps0, a_ps1)
                        acc = accp.tile([128, 1024], F16, tag="acc")
                        for j in range(NT):
                            s_ps = sps.tile([128, 1024], F32, tag="s")
                            for f in range(2):
                                nc.tensor.matmul(
                                    s_ps[:, bass.ts(f, 512)],
                                    kr[:, bass.ts(j, 128)],
                                    qr[:, ihb + f * 512:
                                        ihb + (f + 1) * 512],
                                    start=True, stop=True,
                                )
                            s_exp = sxp.tile([128, 1024], F16, tag="sx")
                            nc.scalar.activation(s_exp[:], s_ps[:], EXP,
                                                 scale=SCALE)
                            if j == 0:
                                nc.vector.tensor_copy(acc[:], s_exp[:])
                            else:
                                nc.vector.tensor_tensor(acc[:], acc[:],
                                                        s_exp[:], op=ADD)
                            for f in range(2):
                                nc.tensor.matmul(
                                    a_ps[f][:],
                                    vv[:, j, bass.ts(h, 128)],
                                    s_exp[:, bass.ts(f, 512)],
                                    start=(j == 0), stop=(j == NT - 1),
                                )
                        # softmax denominators: the all-ones [128,128]
                        # matmul partition-reduces acc AND broadcasts
                        # den[i] to every partition; fast approx
                        # reciprocal; normalize on DVE.
                        for f in range(2):
                            den_ps = pp.tile([128, 512], F32, tag="pp")
                            nc.tensor.matmul(den_ps[:], onem[:],
                                             acc[:, bass.ts(f, 512)],
                                             start=True, stop=True)
                            bc_sb = accp.tile([128, 512], F32, tag="bc")
                            with nc.allow_low_precision(
                                    reason="softmax denominators: approx "
                                           "recip is ~51 ULP"):
                                nc.vector.reciprocal_approx_fast(
                                    out=bc_sb[:], in_=den_ps[:])
                            nc.vector.tensor_tensor(
                                ao[:, h, ihb + f * 512:
                                   ihb + (f + 1) * 512],
                                a_ps[f][:], bc_sb[:],
                                op=MULT)

            # ---- o_proj (wo/st pools reuse the closed wp space) ------
            with (
                tc.tile_pool(name="op", bufs=1) as op,
                tc.tile_pool(name="stp", bufs=3) as stp,
            ):
                wo_sb = op.tile([128, HL, D], F16, tag="wo")
                nc.sync.dma_start(wo_sb[:], wo_d[:])
                for m in range(NT):
                    st = stp.tile([128, D], F16, tag="st")
                    for f in range(4):
                        # spread o_ps over the pp pool and the (now
                        # idle) scores pool for 4-deep PSUM pipelining
                        if f % 2 == 0:
                            o_ps = pp.tile([128, 512], F32, tag="pp")
                        else:
                            o_ps = sps.tile([128, 512], F32, tag="s")
                        for ct in range(HL):
                            nc.tensor.matmul(
                                o_ps[:],
                                ao[:, ct, bass.ts(m, 128)],
                                wo_sb[:, ct, bass.ts(f, 512)],
                                start=(ct == 0), stop=(ct == HL - 1),
                            )
                        # alternate eviction engines so neither paces
                        # o_proj
                        if f % 2 == 0:
                            nc.scalar.copy(st[:, bass.ts(f, 512)],
                                           o_ps[:])
                        else:
                            nc.vector.tensor_copy(st[:, bass.ts(f, 512)],
                                                  o_ps[:])
                            # store each 1024-col half as soon as its
                            # two evictions land (shrinks the tail)
                            nc.sync.dma_start(
                                out_d[bass.ts(m, 128),
                                      (f - 1) * 512:(f + 1) * 512],
                                st[:, (f - 1) * 512:(f + 1) * 512])

    nc.compile()
    return nc


def _rope_tables():
    inv_freq = 1.0 / (10000.0 ** (np.arange(0, HD, 2, dtype=np.float32) / HD))
    pos = np.arange(N, dtype=np.float32)
    freqs = pos[:, None] * inv_freq[None, :]          # [N, HD/2]
    emb = np.concatenate([freqs, freqs], axis=-1)     # [N, HD]
    cos = np.cos(emb).astype(np.float32).T.copy()     # [HD, N]
    sin = np.sin(emb).astype(np.float32).T.copy()     # [HD, N]
    sin_signed = sin.copy()
    sin_signed[0:64] *= -1.0
    return cos, sin_signed


def _make_in_maps(x, Wq, Wk, Wv, Wo):
    cos, sin_signed = _rope_tables()
    f16 = np.float16

    in_maps = []
    for c in range(N_CORES):
        b, hg = c // 4, c % 4
        cols = slice(C * hg, C * hg + C)
        xT = np.ascontiguousarray(x[b].T)                      # [D, N]
        xt = np.ascontiguousarray(
            xT.reshape(KT, 128, NB, 512).transpose(2, 1, 0, 3)
        ).astype(f16)                                          # [NB,128,KT,512]

        def wslice(W):
            wt = W[cols, :].T                                  # [D, C]
            return np.ascontiguousarray(
                wt.reshape(KT, 128, C).transpose(1, 0, 2)
            ).astype(f16)                                      # [128, KT, C]

        wo_t = Wo[:, cols].T                                   # [C, D]
        wo = np.ascontiguousarray(
            wo_t.reshape(HL, 128, D).transpose(1, 0, 2)
        ).astype(f16)                                          # [128, HL, D]

        in_maps.append({
            "xt": xt,
            "wq": wslice(Wq),
            "wk": wslice(Wk),
            "wv": wslice(Wv),
            "wo": wo,
            "cos": cos.astype(f16),
            "sin": sin_signed.astype(f16),
            "onem": np.ones((128, 128), dtype=f16),
        })
    return in_maps


def kernel(x, Wq, Wk, Wv, Wo):
    x = np.asarray(x, dtype=np.float32)
    Wq = np.asarray(Wq, dtype=np.float32)
    Wk = np.asarray(Wk, dtype=np.float32)
    Wv = np.asarray(Wv, dtype=np.float32)
    Wo = np.asarray(Wo, dtype=np.float32)

    if "nc" not in _CACHE:
        _CACHE["nc"] = _build_program()
    nc = _CACHE["nc"]

    in_maps = _make_in_maps(x, Wq, Wk, Wv, Wo)
    results = run_bass_kernel_spmd(
        nc, in_maps, core_ids=list(range(N_CORES))
    ).results

    out = np.zeros((B, N, D), dtype=np.float32)
    for c in range(N_CORES):
        out[c // 4] += results[c]["out"].astype(np.float32)
    return out


# revision 18
# speedup vs baseline: 2.1215x; 1.0677x over previous
"""MHSA + RoPE kernel for Trainium2, 8 NeuronCores.

Sharding: data-parallel over batch (B=2) x tensor-parallel over heads
(16 heads -> 4 head-groups of 4). Core c handles batch c//4, heads
[4*(c%4) : 4*(c%4)+4]. Each core computes its partial o_proj output
[N, D]; host sums the 4 partials per batch (the "all-reduce").

Per-core schedule (single TileContext scope, per-head pipeline so the
Tile scheduler can fill attention's ACT-bound PE gaps with the next
head's projection matmuls and keep the PE HAM clock gate warm):

  h=0: k0,q0 proj (+inline RoPE chunks) -> v proj (all heads) -> attn0
  h>0: k_h,q_h proj + RoPE (overlaps attn_{h-1}) -> attn_h
  o_proj at the end (overlaps attn3 via the scheduler).

RoPE is applied to [128,1024] column chunks right after the projection
eviction that produces them, so the rope->scores dependency chain at a
head boundary is ~2us instead of ~10us.

Softmax denominators: an all-ones [128,128] matmul partition-reduces
acc AND broadcasts the result to all partitions in one shot; the
reciprocal runs as reciprocal_approx_fast (single DVE uop chain, ~5x
faster than reciprocal()); the normalize multiply reads a_ps (PSUM) x
bc (SBUF) on DVE.

Everything on-chip is fp16 (same PE rate as bf16, 2x DVE mode, half
the SBUF/DMA of f32, 11-bit mantissa: exp values <= ~200 and softmax
denominators ~3e3 are represented to ~0.05%). PSUM stays f32.

PSUM budget (8 banks): scores [128,1024]x2 bufs = 4, PV accumulator
[128,1024]x1 = 2, shared proj/tail/o_proj pool [128,512]x2 = 2.
o_proj additionally reuses the scores pool slots once attention ends.
"""

import sys

sys.path.insert(0, "/opt/trn_rl_repo")

import numpy as np

import concourse.bass as bass
import concourse.tile as tile
from concourse import bacc, mybir
from concourse.bass_utils import run_bass_kernel_spmd

F32 = mybir.dt.float32
F16 = mybir.dt.float16
MULT = mybir.AluOpType.mult
ADD = mybir.AluOpType.add
EXP = mybir.ActivationFunctionType.Exp
PSUM = bass.MemorySpace.PSUM

B, N, D = 2, 2048, 2048
H, HD = 16, 128
HL = 4            # local heads per core
C = HL * HD       # 512 local head cols
KT = D // 128     # 16 contraction tiles
NB = 4            # n-blocks of 512 for projections
NT = N // 128     # 16 j-tiles
SCALE = float(HD) ** -0.5
N_CORES = 8

_CACHE = {}


def _build_program():
    nc = bacc.Bacc("TRN2", target_bir_lowering=False, debug=False,
                   num_devices=N_CORES)

    xt_d = nc.dram_tensor("xt", [NB, 128, KT, 512], F16, kind="ExternalInput")
    wq_d = nc.dram_tensor("wq", [128, KT, C], F16, kind="ExternalInput")
    wk_d = nc.dram_tensor("wk", [128, KT, C], F16, kind="ExternalInput")
    wv_d = nc.dram_tensor("wv", [128, KT, C], F16, kind="ExternalInput")
    wo_d = nc.dram_tensor("wo", [128, HL, D], F16, kind="ExternalInput")
    cos_d = nc.dram_tensor("cos", [128, N], F16, kind="ExternalInput")
    sin_d = nc.dram_tensor("sin", [128, N], F16, kind="ExternalInput")
    onem_d = nc.dram_tensor("onem", [128, 128], F16, kind="ExternalInput")
    out_d = nc.dram_tensor("out", [N, D], F16, kind="ExternalOutput")

    with tile.TileContext(nc) as tc:
        with (
            tc.tile_pool(name="res", bufs=1) as res,
            tc.tile_pool(name="qk", bufs=2) as qkp,
            tc.tile_pool(name="rope", bufs=2) as ropep,
            tc.tile_pool(name="sx", bufs=4) as sxp,
            tc.tile_pool(name="accp", bufs=2) as accp,
            tc.tile_pool(name="pp", bufs=2, space=PSUM) as pp,
            tc.tile_pool(name="sps", bufs=2, space=PSUM) as sps,
            tc.tile_pool(name="aps", bufs=2, space=PSUM) as aps,
        ):
            vv = res.tile([128, NT, C], F16)      # v natural [n, c]
            ao = res.tile([128, HL, N], F16)      # normalized A^T [c, n]
            cos_sb = res.tile([128, N], F16)
            sin_sb = res.tile([128, N], F16)
            onem = res.tile([128, 128], F16)

            def rope_chunk(dst, lo):
                # in-place RoPE on dst[:, lo:lo+1024]; sin sign-folded
                # on host. The d-half swap is a partition shuffle ->
                # SBUF-SBUF DMA.
                sl = slice(lo, lo + 1024)
                tmp = ropep.tile([128, 1024], F16, tag="tmp")
                nc.sync.dma_start(tmp[0:64, :], dst[64:128, sl])
                nc.sync.dma_start(tmp[64:128, :], dst[0:64, sl])
                nc.vector.tensor_tensor(tmp[:], tmp[:], sin_sb[:, sl],
                                        op=MULT)
                nc.vector.tensor_tensor(dst[:, sl], dst[:, sl],
                                        cos_sb[:, sl], op=MULT)
                nc.vector.tensor_tensor(dst[:, sl], dst[:, sl], tmp[:],
                                        op=ADD)

            with tc.tile_pool(name="wp", bufs=1) as wp:
                x_sb = wp.tile([128, NB, KT, 512], F16, tag="x")
                wq_sb = wp.tile([128, KT, C], F16, tag="wq")
                wk_sb = wp.tile([128, KT, C], F16, tag="wk")
                wv_sb = wp.tile([128, KT, C], F16, tag="wv")

                # DMA order matches consumption order (k01, q01, v
                # first half, k23, q23, v rest). Half-tensor (1MB)
                # transfers: small per-ktile pieces measured ~200GB/s
                # vs ~430GB/s for large ones, so split no finer than
                # halves, interleaved to spread across DMA queues.
                nc.sync.dma_start(wk_sb[:, 0:8], wk_d[:, 0:8])
                nc.sync.dma_start(x_sb[:, 0, 0:8], xt_d[0, :, 0:8])
                nc.sync.dma_start(wk_sb[:, 8:16], wk_d[:, 8:16])
                nc.sync.dma_start(x_sb[:, 0, 8:16], xt_d[0, :, 8:16])
                nc.sync.dma_start(x_sb[:, 1, 0:8], xt_d[1, :, 0:8])
                nc.sync.dma_start(x_sb[:, 1, 8:16], xt_d[1, :, 8:16])
                nc.sync.dma_start(wq_sb[:, 0:8], wq_d[:, 0:8])
                nc.sync.dma_start(wq_sb[:, 8:16], wq_d[:, 8:16])
                nc.sync.dma_start(wv_sb[:, 0:8], wv_d[:, 0:8])
                nc.sync.dma_start(wv_sb[:, 8:16], wv_d[:, 8:16])
                nc.sync.dma_start(cos_sb[:], cos_d[:])
                nc.sync.dma_start(sin_sb[:], sin_d[:])
                nc.sync.dma_start(onem[:], onem_d[:])
                nc.sync.dma_start(x_sb[:, 2, 0:8], xt_d[2, :, 0:8])
                nc.sync.dma_start(x_sb[:, 2, 8:16], xt_d[2, :, 8:16])
                nc.sync.dma_start(x_sb[:, 3, 0:8], xt_d[3, :, 0:8])
                nc.sync.dma_start(x_sb[:, 3, 8:16], xt_d[3, :, 8:16])

                # Warm the ACT exp table (~2.7us) during the startup
                # DMAs so the first attention exp doesn't eat the load.
                warm = sxp.tile([128, 128], F16, tag="sx")
                nc.scalar.activation(warm[:], cos_sb[:, 0:128], EXP)

                for h in range(HL):
                    # ---- k/q projections for head h: k^T/q^T [d, n],
                    # RoPE chunks inline after the evictions that
                    # complete each 1024-column half. k first so scores
                    # j-tiles unblock as early as possible.
                    qr = qkp.tile([128, N], F16, tag="qr")
                    kr = qkp.tile([128, N], F16, tag="kr")
                    if h == 0:
                        # startup: order matches DMA arrival (wk+x0,
                        # x1, wq, wv, x2, x3) so the PE is never
                        # waiting on a transfer it doesn't need yet,
                        # and attention h0 (which needs kr/qr chunk 0
                        # roped + vv[0..7]) can start early.
                        order = ([("qk", kr, wk_sb, 0),
                                  ("qk", kr, wk_sb, 1),
                                  ("qk", qr, wq_sb, 0),
                                  ("qk", qr, wq_sb, 1)]
                                 + [("v", m) for m in range(8)]
                                 + [("qk", kr, wk_sb, 2),
                                    ("qk", kr, wk_sb, 3),
                                    ("qk", qr, wq_sb, 2),
                                    ("qk", qr, wq_sb, 3)]
                                 + [("v", m) for m in range(8, NT)])
                    else:
                        order = [("qk", kr, wk_sb, nb)
                                 for nb in range(NB)] + \
                                [("qk", qr, wq_sb, nb)
                                 for nb in range(NB)]
                    for item in order:
                        if item[0] == "qk":
                            _, dst, w_sb, nb = item
                            ps = pp.tile([128, 512], F32, tag="pp")
                            for t in range(KT):
                                nc.tensor.matmul(
                                    ps[:],
                                    w_sb[:, t, bass.ts(h, 128)],
                                    x_sb[:, nb, t, :],
                                    start=(t == 0), stop=(t == KT - 1),
                                )
                            nc.scalar.copy(dst[:, bass.ts(nb, 512)],
                                           ps[:])
                            if nb % 2 == 1:
                                rope_chunk(dst, (nb - 1) * 512)
                        else:
                            # ---- v projection, all heads: v [n, c] ---
                            _, m = item
                            nb, mm = m // 4, m % 4
                            ps = pp.tile([128, 512], F32, tag="pp")
                            for t in range(KT):
                                nc.tensor.matmul(
                                    ps[:],
                                    x_sb[:, nb, t, bass.ts(mm, 128)],
                                    wv_sb[:, t, :],
                                    start=(t == 0), stop=(t == KT - 1),
                                )
                            nc.scalar.copy(vv[:, m, :], ps[:])

                    # ---- attention for head h ------------------------
                    for ih in range(2):
                        ihb = ih * 1024
                        # per-512-col accumulator tiles (1 bank each,
                        # 2 bufs): normalize of f=0 can release its
                        # bank while f=1 still accumulates, halving the
                        # ih-boundary WAR stall.
                        a_ps0 = aps.tile([128, 512], F32, tag="a")
                        a_ps1 = aps.tile([128, 512], F32, tag="a")
                        a_ps = (a_ps0, a_ps1)
                        acc = accp.tile([128, 1024], F16, tag="acc")
                        for j in range(NT):
                            s_ps = sps.tile([128, 1024], F32, tag="s")
                            for f in range(2):
                                nc.tensor.matmul(
                                    s_ps[:, bass.ts(f, 512)],
                                    kr[:, bass.ts(j, 128)],
                                    qr[:, ihb + f * 512:
                                        ihb + (f + 1) * 512],
                                    start=True, stop=True,
                                )
                            s_exp = sxp.tile([128, 1024], F16, tag="sx")
                            nc.scalar.activation(s_exp[:], s_ps[:], EXP,
                                                 scale=SCALE)
                            if j == 0:
                                nc.vector.tensor_copy(acc[:], s_exp[:])
                            else:
                                nc.vector.tensor_tensor(acc[:], acc[:],
                                                        s_exp[:], op=ADD)
                            for f in range(2):
                                nc.tensor.matmul(
                                    a_ps[f][:],
                                    vv[:, j, bass.ts(h, 128)],
                                    s_exp[:, bass.ts(f, 512)],
                                    start=(j == 0), stop=(j == NT - 1),
                                )
                        # softmax denominators: the all-ones [128,128]
                        # matmul partition-reduces acc AND broadcasts
                        # den[i] to every partition; fast approx
                        # reciprocal; normalize on DVE.
                        for f in range(2):
                            den_ps = pp.tile([128, 512], F32, tag="pp")
                            nc.tensor.matmul(den_ps[:], onem[:],
                                             acc[:, bass.ts(f, 512)],
                                             start=True, stop=True)
                            bc_sb = accp.tile([128, 512], F32, tag="bc")
                            with nc.allow_low_precision(
                                    reason="softmax denominators: approx "
                                           "recip is ~51 ULP"):
                                nc.vector.reciprocal_approx_fast(
                                    out=bc_sb[:], in_=den_ps[:])
                            nc.vector.tensor_tensor(
                                ao[:, h, ihb + f * 512:
                                   ihb + (f + 1) * 512],
                                a_ps[f][:], bc_sb[:],
                                op=MULT)

            # ---- o_proj (wo/st pools reuse the closed wp space) ------
            with (
                tc.tile_pool(name="op", bufs=1) as op,
                tc.tile_pool(name="stp", bufs=3) as stp,
            ):
                wo_sb = op.tile([128, HL, D], F16, tag="wo")
                nc.sync.dma_start(wo_sb[:], wo_d[:])
                for m in range(NT):
                    st = stp.tile([128, D], F16, tag="st")
                    for f in range(4):
                        # spread o_ps over the pp pool and the (now
                        # idle) scores pool for 4-deep PSUM pipelining
                        if f % 2 == 0:
                            o_ps = pp.tile([128, 512], F32, tag="pp")
                        else:
                            o_ps = sps.tile([128, 512], F32, tag="s")
                        for ct in range(HL):
                            nc.tensor.matmul(
                                o_ps[:],
                                ao[:, ct, bass.ts(m, 128)],
                                wo_sb[:, ct, bass.ts(f, 512)],
                                start=(ct == 0), stop=(ct == HL - 1),
                            )
                        # alternate eviction engines so neither paces
                        # o_proj
                        if f % 2 == 0:
                            nc.scalar.copy(st[:, bass.ts(f, 512)],
                                           o_ps[:])
                        else:
                            nc.vector.tensor_copy(st[:, bass.ts(f, 512)],
                                                  o_ps[:])
                            # store each 1024-col half as soon as its
                            # two evictions land (shrinks the tail)
                            nc.sync.dma_start(
                                out_d[bass.ts(m, 128),
                                      (f - 1) * 512:(f + 1) * 512],
                                st[:, (f - 1) * 512:(f + 1) * 512])

    nc.compile()
    return nc


def _rope_tables():
    inv_freq = 1.0 / (10000.0 ** (np.arange(0, HD, 2, dtype=np.float32) / HD))
    pos = np.arange(N, dtype=np.float32)
    freqs = pos[:, None] * inv_freq[None, :]          # [N, HD/2]
    emb = np.concatenate([freqs, freqs], axis=-1)     # [N, HD]
    cos = np.cos(emb).astype(np.float32).T.copy()     # [HD, N]
    sin = np.sin(emb).astype(np.float32).T.copy()     # [HD, N]
    sin_signed = sin.copy()
    sin_signed[0:64] *= -1.0
    return cos, sin_signed


def _make_in_maps(x, Wq, Wk, Wv, Wo):
    cos, sin_signed = _rope_tables()
    f16 = np.float16

    in_maps = []
    for c in range(N_CORES):
        b, hg = c // 4, c % 4
        cols = slice(C * hg, C * hg + C)
        xT = np.ascontiguousarray(x[b].T)                      # [D, N]
        xt = np.ascontiguousarray(
            xT.reshape(KT, 128, NB, 512).transpose(2, 1, 0, 3)
        ).astype(f16)                                          # [NB,128,KT,512]

        def wslice(W):
            wt = W[cols, :].T                                  # [D, C]
            return np.ascontiguousarray(
                wt.reshape(KT, 128, C).transpose(1, 0, 2)
            ).astype(f16)                                      # [128, KT, C]

        wo_t = Wo[:, cols].T                                   # [C, D]
        wo = np.ascontiguousarray(
            wo_t.reshape(HL, 128, D).transpose(1, 0, 2)
        ).astype(f16)                                          # [128, HL, D]

        in_maps.append({
            "xt": xt,
            "wq": wslice(Wq),
            "wk": wslice(Wk),
            "wv": wslice(Wv),
            "wo": wo,
            "cos": cos.astype(f16),
            "sin": sin_signed.astype(f16),
            "onem": np.ones((128, 128), dtype=f16),
        })
    return in_maps


def kernel(x, Wq, Wk, Wv, Wo):
    x = np.asarray(x, dtype=np.float32)
    Wq = np.asarray(Wq, dtype=np.float32)
    Wk = np.asarray(Wk, dtype=np.float32)
    Wv = np.asarray(Wv, dtype=np.float32)
    Wo = np.asarray(Wo, dtype=np.float32)

    if "nc" not in _CACHE:
        _CACHE["nc"] = _build_program()
    nc = _CACHE["nc"]

    in_maps = _make_in_maps(x, Wq, Wk, Wv, Wo)
    results = run_bass_kernel_spmd(
        nc, in_maps, core_ids=list(range(N_CORES))
    ).results

    out = np.zeros((B, N, D), dtype=np.float32)
    for c in range(N_CORES):
        out[c // 4] += results[c]["out"].astype(np.float32)
    return out


# revision 19
# speedup vs baseline: 2.1231x; 1.0007x over previous
"""MHSA + RoPE kernel for Trainium2, 8 NeuronCores.

Sharding: data-parallel over batch (B=2) x tensor-parallel over heads
(16 heads -> 4 head-groups of 4). Core c handles batch c//4, heads
[4*(c%4) : 4*(c%4)+4]. Each core computes its partial o_proj output
[N, D]; host sums the 4 partials per batch (the "all-reduce").

Per-core schedule (single TileContext scope, per-head pipeline so the
Tile scheduler can fill attention's ACT-bound PE gaps with the next
head's projection matmuls and keep the PE HAM clock gate warm):

  h=0: k0,q0 proj (+inline RoPE chunks) -> v proj (all heads) -> attn0
  h>0: k_h,q_h proj + RoPE (overlaps attn_{h-1}) -> attn_h
  o_proj at the end (overlaps attn3 via the scheduler).

RoPE is applied to [128,1024] column chunks right after the projection
eviction that produces them, so the rope->scores dependency chain at a
head boundary is ~2us instead of ~10us.

Softmax denominators: an all-ones [128,128] matmul partition-reduces
acc AND broadcasts the result to all partitions in one shot; the
reciprocal runs as reciprocal_approx_fast (single DVE uop chain, ~5x
faster than reciprocal()); the normalize multiply reads a_ps (PSUM) x
bc (SBUF) on DVE.

Everything on-chip is fp16 (same PE rate as bf16, 2x DVE mode, half
the SBUF/DMA of f32, 11-bit mantissa: exp values <= ~200 and softmax
denominators ~3e3 are represented to ~0.05%). PSUM stays f32.

PSUM budget (8 banks): scores [128,1024]x2 bufs = 4, PV accumulator
[128,1024]x1 = 2, shared proj/tail/o_proj pool [128,512]x2 = 2.
o_proj additionally reuses the scores pool slots once attention ends.
"""

import sys

sys.path.insert(0, "/opt/trn_rl_repo")

import numpy as np

import concourse.bass as bass
import concourse.tile as tile
from concourse import bacc, mybir
from concourse.bass_utils import run_bass_kernel_spmd

F32 = mybir.dt.float32
F16 = mybir.dt.float16
MULT = mybir.AluOpType.mult
ADD = mybir.AluOpType.add
EXP = mybir.ActivationFunctionType.Exp
PSUM = bass.MemorySpace.PSUM

B, N, D = 2, 2048, 2048
H, HD = 16, 128
HL = 4            # local heads per core
C = HL * HD       # 512 local head cols
KT = D // 128     # 16 contraction tiles
NB = 4            # n-blocks of 512 for projections
NT = N // 128     # 16 j-tiles
SCALE = float(HD) ** -0.5
N_CORES = 8

_CACHE = {}


def _build_program():
    nc = bacc.Bacc("TRN2", target_bir_lowering=False, debug=False,
                   num_devices=N_CORES)

    xt_d = nc.dram_tensor("xt", [NB, 128, KT, 512], F16, kind="ExternalInput")
    wq_d = nc.dram_tensor("wq", [128, KT, C], F16, kind="ExternalInput")
    wk_d = nc.dram_tensor("wk", [128, KT, C], F16, kind="ExternalInput")
    wv_d = nc.dram_tensor("wv", [128, KT, C], F16, kind="ExternalInput")
    wo_d = nc.dram_tensor("wo", [128, HL, D], F16, kind="ExternalInput")
    cos_d = nc.dram_tensor("cos", [128, N], F16, kind="ExternalInput")
    sin_d = nc.dram_tensor("sin", [128, N], F16, kind="ExternalInput")
    onem_d = nc.dram_tensor("onem", [128, 128], F16, kind="ExternalInput")
    out_d = nc.dram_tensor("out", [N, D], F16, kind="ExternalOutput")

    with tile.TileContext(nc) as tc:
        with (
            tc.tile_pool(name="res", bufs=1) as res,
            tc.tile_pool(name="qk", bufs=2) as qkp,
            tc.tile_pool(name="rope", bufs=2) as ropep,
            tc.tile_pool(name="sx", bufs=4) as sxp,
            tc.tile_pool(name="accp", bufs=2) as accp,
            tc.tile_pool(name="pp", bufs=2, space=PSUM) as pp,
            tc.tile_pool(name="sps", bufs=2, space=PSUM) as sps,
            tc.tile_pool(name="aps", bufs=2, space=PSUM) as aps,
        ):
            vv = res.tile([128, NT, C], F16)      # v natural [n, c]
            ao = res.tile([128, HL, N], F16)      # normalized A^T [c, n]
            cos_sb = res.tile([128, N], F16)
            sin_sb = res.tile([128, N], F16)
            onem = res.tile([128, 128], F16)

            def rope_chunk(dst, lo):
                # in-place RoPE on dst[:, lo:lo+1024]; sin sign-folded
                # on host. The d-half swap is a partition shuffle ->
                # SBUF-SBUF DMA.
                sl = slice(lo, lo + 1024)
                tmp = ropep.tile([128, 1024], F16, tag="tmp")
                nc.sync.dma_start(tmp[0:64, :], dst[64:128, sl])
                nc.sync.dma_start(tmp[64:128, :], dst[0:64, sl])
                nc.vector.tensor_tensor(tmp[:], tmp[:], sin_sb[:, sl],
                                        op=MULT)
                nc.vector.tensor_tensor(dst[:, sl], dst[:, sl],
                                        cos_sb[:, sl], op=MULT)
                nc.vector.tensor_tensor(dst[:, sl], dst[:, sl], tmp[:],
                                        op=ADD)

            with tc.tile_pool(name="wp", bufs=1) as wp:
                x_sb = wp.tile([128, NB, KT, 512], F16, tag="x")
                wq_sb = wp.tile([128, KT, C], F16, tag="wq")
                wk_sb = wp.tile([128, KT, C], F16, tag="wk")
                wv_sb = wp.tile([128, KT, C], F16, tag="wv")

                # DMA order matches consumption order (k01, q01, v
                # first half, k23, q23, v rest). Half-tensor (1MB)
                # transfers: small per-ktile pieces measured ~200GB/s
                # vs ~430GB/s for large ones, so split no finer than
                # halves, interleaved to spread across DMA queues.
                nc.sync.dma_start(wk_sb[:, 0:8], wk_d[:, 0:8])
                nc.sync.dma_start(x_sb[:, 0, 0:8], xt_d[0, :, 0:8])
                nc.sync.dma_start(wk_sb[:, 8:16], wk_d[:, 8:16])
                nc.sync.dma_start(x_sb[:, 0, 8:16], xt_d[0, :, 8:16])
                nc.sync.dma_start(x_sb[:, 1, 0:8], xt_d[1, :, 0:8])
                nc.sync.dma_start(x_sb[:, 1, 8:16], xt_d[1, :, 8:16])
                nc.sync.dma_start(wq_sb[:, 0:8], wq_d[:, 0:8])
                nc.sync.dma_start(wq_sb[:, 8:16], wq_d[:, 8:16])
                nc.sync.dma_start(wv_sb[:, 0:8], wv_d[:, 0:8])
                nc.sync.dma_start(wv_sb[:, 8:16], wv_d[:, 8:16])
                nc.sync.dma_start(cos_sb[:], cos_d[:])
                nc.sync.dma_start(sin_sb[:], sin_d[:])
                nc.sync.dma_start(onem[:], onem_d[:])
                nc.sync.dma_start(x_sb[:, 2, 0:8], xt_d[2, :, 0:8])
                nc.sync.dma_start(x_sb[:, 2, 8:16], xt_d[2, :, 8:16])
                nc.sync.dma_start(x_sb[:, 3, 0:8], xt_d[3, :, 0:8])
                nc.sync.dma_start(x_sb[:, 3, 8:16], xt_d[3, :, 8:16])

                # Warm the ACT exp table (~2.7us) during the startup
                # DMAs so the first attention exp doesn't eat the load.
                warm = sxp.tile([128, 128], F16, tag="sx")
                nc.scalar.activation(warm[:], cos_sb[:, 0:128], EXP)

                for h in range(HL):
                    # ---- k/q projections for head h: k^T/q^T [d, n],
                    # RoPE chunks inline after the evictions that
                    # complete each 1024-column half. k first so scores
                    # j-tiles unblock as early as possible.
                    qr = qkp.tile([128, N], F16, tag="qr")
                    kr = qkp.tile([128, N], F16, tag="kr")
                    if h == 0:
                        # startup: order matches DMA arrival (wk+x0,
                        # x1, wq, wv, x2, x3) so the PE is never
                        # waiting on a transfer it doesn't need yet,
                        # and attention h0 (which needs kr/qr chunk 0
                        # roped + vv[0..7]) can start early.
                        order = ([("qk", kr, wk_sb, 0),
                                  ("qk", kr, wk_sb, 1),
                                  ("qk", qr, wq_sb, 0),
                                  ("qk", qr, wq_sb, 1)]
                                 + [("v", m) for m in range(8)]
                                 + [("qk", kr, wk_sb, 2),
                                    ("qk", kr, wk_sb, 3),
                                    ("qk", qr, wq_sb, 2),
                                    ("qk", qr, wq_sb, 3)]
                                 + [("v", m) for m in range(8, NT)])
                    else:
                        order = [("qk", kr, wk_sb, nb)
                                 for nb in range(NB)] + \
                                [("qk", qr, wq_sb, nb)
                                 for nb in range(NB)]
                    for item in order:
                        if item[0] == "qk":
                            _, dst, w_sb, nb = item
                            ps = pp.tile([128, 512], F32, tag="pp")
                            for t in range(KT):
                                nc.tensor.matmul(
                                    ps[:],
                                    w_sb[:, t, bass.ts(h, 128)],
                                    x_sb[:, nb, t, :],
                                    start=(t == 0), stop=(t == KT - 1),
                                )
                            nc.scalar.copy(dst[:, bass.ts(nb, 512)],
                                           ps[:])
                            if nb % 2 == 1:
                                rope_chunk(dst, (nb - 1) * 512)
                        else:
                            # ---- v projection, all heads: v [n, c] ---
                            _, m = item
                            nb, mm = m // 4, m % 4
                            ps = pp.tile([128, 512], F32, tag="pp")
                            for t in range(KT):
                                nc.tensor.matmul(
                                    ps[:],
                                    x_sb[:, nb, t, bass.ts(mm, 128)],
                                    wv_sb[:, t, :],
                                    start=(t == 0), stop=(t == KT - 1),
                                )
                            nc.scalar.copy(vv[:, m, :], ps[:])

                    # ---- attention for head h ------------------------
                    for ih in range(2):
                        ihb = ih * 1024
                        # per-512-col accumulator tiles (1 bank each,
                        # 2 bufs): normalize of f=0 can release its
                        # bank while f=1 still accumulates, halving the
                        # ih-boundary WAR stall.
                        a_ps0 = aps.tile([128, 512], F32, tag="a")
                        a_ps1 = aps.tile([128, 512], F32, tag="a")
                        a_ps = (a_ps0, a_ps1)
                        acc = accp.tile([128, 1024], F16, tag="acc")
                        for j in range(NT):
                            s_ps = sps.tile([128, 1024], F32, tag="s")
                            for f in range(2):
                                nc.tensor.matmul(
                                    s_ps[:, bass.ts(f, 512)],
                                    kr[:, bass.ts(j, 128)],
                                    qr[:, ihb + f * 512:
                                        ihb + (f + 1) * 512],
                                    start=True, stop=True,
                                )
                            s_exp = sxp.tile([128, 1024], F16, tag="sx")
                            nc.scalar.activation(s_exp[:], s_ps[:], EXP,
                                                 scale=SCALE)
                            if j == 0:
                                nc.vector.tensor_copy(acc[:], s_exp[:])
                            else:
                                nc.vector.tensor_tensor(acc[:], acc[:],
                                                        s_exp[:], op=ADD)
                            for f in range(2):
                                nc.tensor.matmul(
                                    a_ps[f][:],
                                    vv[:, j, bass.ts(h, 128)],
                                    s_exp[:, bass.ts(f, 512)],
                                    start=(j == 0), stop=(j == NT - 1),
                                )
                        # softmax denominators: the all-ones [128,128]
                        # matmul partition-reduces acc AND broadcasts
                        # den[i] to every partition; fast approx
                        # reciprocal; normalize on DVE.
                        for f in range(2):
                            den_ps = pp.tile([128, 512], F32, tag="pp")
                            nc.tensor.matmul(den_ps[:], onem[:],
                                             acc[:, bass.ts(f, 512)],
                                             start=True, stop=True)
                            bc_sb = accp.tile([128, 512], F32, tag="bc")
                            with nc.allow_low_precision(
                                    reason="softmax denominators: approx "
                                           "recip is ~51 ULP"):
                                nc.vector.reciprocal_approx_fast(
                                    out=bc_sb[:], in_=den_ps[:])
                            nc.vector.tensor_tensor(
                                ao[:, h, ihb + f * 512:
                                   ihb + (f + 1) * 512],
                                a_ps[f][:], bc_sb[:],
                                op=MULT)

            # ---- o_proj (wo/st pools reuse the closed wp space) ------
            with (
                tc.tile_pool(name="op", bufs=1) as op,
                tc.tile_pool(name="stp", bufs=3) as stp,
            ):
                wo_sb = op.tile([128, HL, D], F16, tag="wo")
                nc.sync.dma_start(wo_sb[:], wo_d[:])
                for m in range(NT):
                    st = stp.tile([128, D], F16, tag="st")
                    for f in range(4):
                        # spread o_ps over the pp pool and the (now
                        # idle) scores pool for 4-deep PSUM pipelining
                        if f % 2 == 0:
                            o_ps = pp.tile([128, 512], F32, tag="pp")
                        else:
                            o_ps = sps.tile([128, 512], F32, tag="s")
                        for ct in range(HL):
                            nc.tensor.matmul(
                                o_ps[:],
                                ao[:, ct, bass.ts(m, 128)],
                                wo_sb[:, ct, bass.ts(f, 512)],
                                start=(ct == 0), stop=(ct == HL - 1),
                            )
                        # alternate eviction engines so neither paces
                        # o_proj
                        if f % 2 == 0:
                            nc.scalar.copy(st[:, bass.ts(f, 512)],
                                           o_ps[:])
                        else:
                            nc.vector.tensor_copy(st[:, bass.ts(f, 512)],
                                                  o_ps[:])
                        # store each 512-col slice as soon as its
                        # eviction lands (shrinks the final-store tail)
                        nc.sync.dma_start(
                            out_d[bass.ts(m, 128), bass.ts(f, 512)],
                            st[:, bass.ts(f, 512)])

    nc.compile()
    return nc


def _rope_tables():
    inv_freq = 1.0 / (10000.0 ** (np.arange(0, HD, 2, dtype=np.float32) / HD))
    pos = np.arange(N, dtype=np.float32)
    freqs = pos[:, None] * inv_freq[None, :]          # [N, HD/2]
    emb = np.concatenate([freqs, freqs], axis=-1)     # [N, HD]
    cos = np.cos(emb).astype(np.float32).T.copy()     # [HD, N]
    sin = np.sin(emb).astype(np.float32).T.copy()     # [HD, N]
    sin_signed = sin.copy()
    sin_signed[0:64] *= -1.0
    return cos, sin_signed


def _make_in_maps(x, Wq, Wk, Wv, Wo):
    cos, sin_signed = _rope_tables()
    f16 = np.float16

    in_maps = []
    for c in range(N_CORES):
        b, hg = c // 4, c % 4
        cols = slice(C * hg, C * hg + C)
        xT = np.ascontiguousarray(x[b].T)                      # [D, N]
        xt = np.ascontiguousarray(
            xT.reshape(KT, 128, NB, 512).transpose(2, 1, 0, 3)
        ).astype(f16)                                          # [NB,128,KT,512]

        def wslice(W):
            wt = W[cols, :].T                                  # [D, C]
            return np.ascontiguousarray(
                wt.reshape(KT, 128, C).transpose(1, 0, 2)
            ).astype(f16)                                      # [128, KT, C]

        wo_t = Wo[:, cols].T                                   # [C, D]
        wo = np.ascontiguousarray(
            wo_t.reshape(HL, 128, D).transpose(1, 0, 2)
        ).astype(f16)                                          # [128, HL, D]

        in_maps.append({
            "xt": xt,
            "wq": wslice(Wq),
            "wk": wslice(Wk),
            "wv": wslice(Wv),
            "wo": wo,
            "cos": cos.astype(f16),
            "sin": sin_signed.astype(f16),
            "onem": np.ones((128, 128), dtype=f16),
        })
    return in_maps


def kernel(x, Wq, Wk, Wv, Wo):
    x = np.asarray(x, dtype=np.float32)
    Wq = np.asarray(Wq, dtype=np.float32)
    Wk = np.asarray(Wk, dtype=np.float32)
    Wv = np.asarray(Wv, dtype=np.float32)
    Wo = np.asarray(Wo, dtype=np.float32)

    if "nc" not in _CACHE:
        _CACHE["nc"] = _build_program()
    nc = _CACHE["nc"]

    in_maps = _make_in_maps(x, Wq, Wk, Wv, Wo)
    results = run_bass_kernel_spmd(
        nc, in_maps, core_ids=list(range(N_CORES))
    ).results

    out = np.zeros((B, N, D), dtype=np.float32)
    for c in range(N_CORES):
        out[c // 4] += results[c]["out"].astype(np.float32)
    return out


# revision 20
# speedup vs baseline: 2.2670x; 1.0678x over previous
"""MHSA + RoPE kernel for Trainium2, 8 NeuronCores.

Sharding: data-parallel over batch (B=2) x tensor-parallel over heads
(16 heads -> 4 head-groups of 4). Core c handles batch c//4, heads
[4*(c%4) : 4*(c%4)+4]. Each core computes its partial o_proj output
[N, D]; host sums the 4 partials per batch (the "all-reduce").

Per-core schedule (single TileContext scope, per-head pipeline so the
Tile scheduler can fill attention's ACT-bound PE gaps with the next
head's projection matmuls and keep the PE HAM clock gate warm):

  h=0: k0,q0 proj (+inline RoPE chunks) -> v proj (all heads) -> attn0
  h>0: k_h,q_h proj + RoPE (overlaps attn_{h-1}) -> attn_h
  o_proj at the end (overlaps attn3 via the scheduler).

RoPE is applied to [128,1024] column chunks right after the projection
eviction that produces them, so the rope->scores dependency chain at a
head boundary is ~2us instead of ~10us.

Softmax denominators: an all-ones [128,128] matmul partition-reduces
acc AND broadcasts the result to all partitions in one shot; the
reciprocal runs as reciprocal_approx_fast (single DVE uop chain, ~5x
faster than reciprocal()); the normalize multiply reads a_ps (PSUM) x
bc (SBUF) on DVE.

Everything on-chip is fp16 (same PE rate as bf16, 2x DVE mode, half
the SBUF/DMA of f32, 11-bit mantissa: exp values <= ~200 and softmax
denominators ~3e3 are represented to ~0.05%). PSUM stays f32.

PSUM budget (8 banks): scores [128,1024]x2 bufs = 4, PV accumulator
[128,1024]x1 = 2, shared proj/tail/o_proj pool [128,512]x2 = 2.
o_proj additionally reuses the scores pool slots once attention ends.
"""

import sys

sys.path.insert(0, "/opt/trn_rl_repo")

import numpy as np

import concourse.bass as bass
import concourse.tile as tile
from concourse import bacc, mybir
from concourse.bass_utils import run_bass_kernel_spmd

F32 = mybir.dt.float32
F16 = mybir.dt.float16
MULT = mybir.AluOpType.mult
ADD = mybir.AluOpType.add
EXP = mybir.ActivationFunctionType.Exp
PSUM = bass.MemorySpace.PSUM

B, N, D = 2, 2048, 2048
H, HD = 16, 128
HL = 4            # local heads per core
C = HL * HD       # 512 local head cols
KT = D // 128     # 16 contraction tiles
NB = 4            # n-blocks of 512 for projections
NT = N // 128     # 16 j-tiles
SCALE = float(HD) ** -0.5
N_CORES = 8

_CACHE = {}


def _build_program():
    nc = bacc.Bacc("TRN2", target_bir_lowering=False, debug=False,
                   num_devices=N_CORES)

    xt_d = nc.dram_tensor("xt", [NB, 128, KT, 512], F16, kind="ExternalInput")
    wq_d = nc.dram_tensor("wq", [128, KT, C], F16, kind="ExternalInput")
    wk_d = nc.dram_tensor("wk", [128, KT, C], F16, kind="ExternalInput")
    wv_d = nc.dram_tensor("wv", [128, KT, C], F16, kind="ExternalInput")
    wo_d = nc.dram_tensor("wo", [128, HL, D], F16, kind="ExternalInput")
    cos_d = nc.dram_tensor("cos", [128, N], F16, kind="ExternalInput")
    sin_d = nc.dram_tensor("sin", [128, N], F16, kind="ExternalInput")
    onem_d = nc.dram_tensor("onem", [128, 128], F16, kind="ExternalInput")
    out_d = nc.dram_tensor("out", [N, D], F16, kind="ExternalOutput")

    with tile.TileContext(nc) as tc:
        with (
            tc.tile_pool(name="res", bufs=1) as res,
            tc.tile_pool(name="qk", bufs=2) as qkp,
            tc.tile_pool(name="rope", bufs=2) as ropep,
            tc.tile_pool(name="sx", bufs=4) as sxp,
            tc.tile_pool(name="accp", bufs=2) as accp,
            tc.tile_pool(name="pp", bufs=2, space=PSUM) as pp,
            tc.tile_pool(name="sps", bufs=2, space=PSUM) as sps,
            tc.tile_pool(name="aps", bufs=2, space=PSUM) as aps,
        ):
            vv = res.tile([128, NT, C], F16)      # v natural [n, c]
            ao = res.tile([128, HL, N], F16)      # normalized A^T [c, n]
            cos_sb = res.tile([128, N], F16)
            sin_sb = res.tile([128, N], F16)
            onem = res.tile([128, 128], F16)

            def rope_chunk(dst, lo):
                # in-place RoPE on dst[:, lo:lo+1024]; sin sign-folded
                # on host. The d-half swap is a partition shuffle ->
                # SBUF-SBUF DMA.
                sl = slice(lo, lo + 1024)
                tmp = ropep.tile([128, 1024], F16, tag="tmp")
                nc.sync.dma_start(tmp[0:64, :], dst[64:128, sl])
                nc.sync.dma_start(tmp[64:128, :], dst[0:64, sl])
                nc.vector.tensor_tensor(tmp[:], tmp[:], sin_sb[:, sl],
                                        op=MULT)
                nc.vector.tensor_tensor(dst[:, sl], dst[:, sl],
                                        cos_sb[:, sl], op=MULT)
                nc.vector.tensor_tensor(dst[:, sl], dst[:, sl], tmp[:],
                                        op=ADD)

            with tc.tile_pool(name="wp", bufs=1) as wp:
                x_sb = wp.tile([128, NB, KT, 512], F16, tag="x")
                wq_sb = wp.tile([128, KT, C], F16, tag="wq")
                wk_sb = wp.tile([128, KT, C], F16, tag="wk")
                wv_sb = wp.tile([128, KT, C], F16, tag="wv")

                # DMA order matches consumption order (k01, q01, v
                # first half, k23, q23, v rest). Half-tensor (1MB)
                # transfers: small per-ktile pieces measured ~200GB/s
                # vs ~430GB/s for large ones, so split no finer than
                # halves, interleaved to spread across DMA queues.
                nc.sync.dma_start(wk_sb[:, 0:8], wk_d[:, 0:8])
                nc.sync.dma_start(x_sb[:, 0, 0:8], xt_d[0, :, 0:8])
                nc.sync.dma_start(wk_sb[:, 8:16], wk_d[:, 8:16])
                nc.sync.dma_start(x_sb[:, 0, 8:16], xt_d[0, :, 8:16])
                nc.sync.dma_start(x_sb[:, 1, 0:8], xt_d[1, :, 0:8])
                nc.sync.dma_start(x_sb[:, 1, 8:16], xt_d[1, :, 8:16])
                nc.sync.dma_start(wq_sb[:, 0:8], wq_d[:, 0:8])
                nc.sync.dma_start(wq_sb[:, 8:16], wq_d[:, 8:16])
                nc.sync.dma_start(wv_sb[:, 0:8], wv_d[:, 0:8])
                nc.sync.dma_start(wv_sb[:, 8:16], wv_d[:, 8:16])
                nc.sync.dma_start(cos_sb[:], cos_d[:])
                nc.sync.dma_start(sin_sb[:], sin_d[:])
                nc.sync.dma_start(onem[:], onem_d[:])
                nc.sync.dma_start(x_sb[:, 2, 0:8], xt_d[2, :, 0:8])
                nc.sync.dma_start(x_sb[:, 2, 8:16], xt_d[2, :, 8:16])
                nc.sync.dma_start(x_sb[:, 3, 0:8], xt_d[3, :, 0:8])
                nc.sync.dma_start(x_sb[:, 3, 8:16], xt_d[3, :, 8:16])

                # Warm the ACT exp table (~2.7us) during the startup
                # DMAs so the first attention exp doesn't eat the load.
                warm = sxp.tile([128, 128], F16, tag="sx")
                nc.scalar.activation(warm[:], cos_sb[:, 0:128], EXP)

                for h in range(HL):
                    # ---- k/q projections for head h: k^T/q^T [d, n],
                    # RoPE chunks inline after the evictions that
                    # complete each 1024-column half. k first so scores
                    # j-tiles unblock as early as possible.
                    qr = qkp.tile([128, N], F16, tag="qr")
                    kr = qkp.tile([128, N], F16, tag="kr")
                    if h == 0:
                        # startup: order matches DMA arrival (wk+x0,
                        # x1, wq, wv, x2, x3) so the PE is never
                        # waiting on a transfer it doesn't need yet,
                        # and attention h0 (which needs kr/qr chunk 0
                        # roped + vv[0..7]) can start early.
                        order = ([("qk", kr, wk_sb, 0),
                                  ("qk", kr, wk_sb, 1),
                                  ("qk", qr, wq_sb, 0),
                                  ("qk", qr, wq_sb, 1)]
                                 + [("v", m) for m in range(8)]
                                 + [("qk", kr, wk_sb, 2),
                                    ("qk", kr, wk_sb, 3),
                                    ("qk", qr, wq_sb, 2),
                                    ("qk", qr, wq_sb, 3)]
                                 + [("v", m) for m in range(8, NT)])
                    else:
                        order = [("qk", kr, wk_sb, nb)
                                 for nb in range(NB)] + \
                                [("qk", qr, wq_sb, nb)
                                 for nb in range(NB)]
                    for item in order:
                        if item[0] == "qk":
                            _, dst, w_sb, nb = item
                            ps = pp.tile([128, 512], F32, tag="pp")
                            for t in range(KT):
                                nc.tensor.matmul(
                                    ps[:],
                                    w_sb[:, t, bass.ts(h, 128)],
                                    x_sb[:, nb, t, :],
                                    start=(t == 0), stop=(t == KT - 1),
                                )
                            nc.scalar.copy(dst[:, bass.ts(nb, 512)],
                                           ps[:])
                            if nb % 2 == 1:
                                rope_chunk(dst, (nb - 1) * 512)
                        else:
                            # ---- v projection, all heads: v [n, c] ---
                            _, m = item
                            nb, mm = m // 4, m % 4
                            ps = pp.tile([128, 512], F32, tag="pp")
                            for t in range(KT):
                                nc.tensor.matmul(
                                    ps[:],
                                    x_sb[:, nb, t, bass.ts(mm, 128)],
                                    wv_sb[:, t, :],
                                    start=(t == 0), stop=(t == KT - 1),
                                )
                            nc.scalar.copy(vv[:, m, :], ps[:])

                    # ---- attention for head h ------------------------
                    for ih in range(2):
                        ihb = ih * 1024
                        # per-512-col accumulator tiles (1 bank each,
                        # 2 bufs): normalize of f=0 can release its
                        # bank while f=1 still accumulates, halving the
                        # ih-boundary WAR stall.
                        a_ps0 = aps.tile([128, 512], F32, tag="a")
                        a_ps1 = aps.tile([128, 512], F32, tag="a")
                        a_ps = (a_ps0, a_ps1)
                        acc = accp.tile([128, 1024], F16, tag="acc")
                        for j in range(NT):
                            s_ps = sps.tile([128, 1024], F32, tag="s")
                            for f in range(2):
                                nc.tensor.matmul(
                                    s_ps[:, bass.ts(f, 512)],
                                    kr[:, bass.ts(j, 128)],
                                    qr[:, ihb + f * 512:
                                        ihb + (f + 1) * 512],
                                    start=True, stop=True,
                                )
                            s_exp = sxp.tile([128, 1024], F16, tag="sx")
                            nc.scalar.activation(s_exp[:], s_ps[:], EXP,
                                                 scale=SCALE)
                            if j == 0:
                                nc.vector.tensor_copy(acc[:], s_exp[:])
                            else:
                                nc.vector.tensor_tensor(acc[:], acc[:],
                                                        s_exp[:], op=ADD)
                            for f in range(2):
                                nc.tensor.matmul(
                                    a_ps[f][:],
                                    vv[:, j, bass.ts(h, 128)],
                                    s_exp[:, bass.ts(f, 512)],
                                    start=(j == 0), stop=(j == NT - 1),
                                )
                        # softmax denominators: the all-ones [128,128]
                        # matmul partition-reduces acc AND broadcasts
                        # den[i] to every partition; fast approx
                        # reciprocal; normalize on DVE.
                        for f in range(2):
                            # use a scores-pool slot (fast-cycling, not
                            # the pp slots that next-head proj groups
                            # need to fill the ih-boundary gap)
                            den_ps = sps.tile([128, 512], F32, tag="s")
                            nc.tensor.matmul(den_ps[:], onem[:],
                                             acc[:, bass.ts(f, 512)],
                                             start=True, stop=True)
                            bc_sb = accp.tile([128, 512], F32, tag="bc")
                            with nc.allow_low_precision(
                                    reason="softmax denominators: approx "
                                           "recip is ~51 ULP"):
                                nc.vector.reciprocal_approx_fast(
                                    out=bc_sb[:], in_=den_ps[:])
                            nc.vector.tensor_tensor(
                                ao[:, h, ihb + f * 512:
                                   ihb + (f + 1) * 512],
                                a_ps[f][:], bc_sb[:],
                                op=MULT)

            # ---- o_proj (wo/st pools reuse the closed wp space) ------
            with (
                tc.tile_pool(name="op", bufs=1) as op,
                tc.tile_pool(name="stp", bufs=3) as stp,
            ):
                wo_sb = op.tile([128, HL, D], F16, tag="wo")
                nc.sync.dma_start(wo_sb[:], wo_d[:])
                for m in range(NT):
                    st = stp.tile([128, D], F16, tag="st")
                    for f in range(4):
                        # spread o_ps over the pp pool and the (now
                        # idle) scores pool for 4-deep PSUM pipelining
                        if f % 2 == 0:
                            o_ps = pp.tile([128, 512], F32, tag="pp")
                        else:
                            o_ps = sps.tile([128, 512], F32, tag="s")
                        for ct in range(HL):
                            nc.tensor.matmul(
                                o_ps[:],
                                ao[:, ct, bass.ts(m, 128)],
                                wo_sb[:, ct, bass.ts(f, 512)],
                                start=(ct == 0), stop=(ct == HL - 1),
                            )
                        # alternate eviction engines so neither paces
                        # o_proj
                        if f % 2 == 0:
                            nc.scalar.copy(st[:, bass.ts(f, 512)],
                                           o_ps[:])
                        else:
                            nc.vector.tensor_copy(st[:, bass.ts(f, 512)],
                                                  o_ps[:])
                        # store each 512-col slice as soon as its
                        # eviction lands (shrinks the final-store tail)
                        nc.sync.dma_start(
                            out_d[bass.ts(m, 128), bass.ts(f, 512)],
                            st[:, bass.ts(f, 512)])

    nc.compile()
    return nc


def _rope_tables():
    inv_freq = 1.0 / (10000.0 ** (np.arange(0, HD, 2, dtype=np.float32) / HD))
    pos = np.arange(N, dtype=np.float32)
    freqs = pos[:, None] * inv_freq[None, :]          # [N, HD/2]
    emb = np.concatenate([freqs, freqs], axis=-1)     # [N, HD]
    cos = np.cos(emb).astype(np.float32).T.copy()     # [HD, N]
    sin = np.sin(emb).astype(np.float32).T.copy()     # [HD, N]
    sin_signed = sin.copy()
    sin_signed[0:64] *= -1.0
    return cos, sin_signed


def _make_in_maps(x, Wq, Wk, Wv, Wo):
    cos, sin_signed = _rope_tables()
    f16 = np.float16

    in_maps = []
    for c in range(N_CORES):
        b, hg = c // 4, c % 4
        cols = slice(C * hg, C * hg + C)
        xT = np.ascontiguousarray(x[b].T)                      # [D, N]
        xt = np.ascontiguousarray(
            xT.reshape(KT, 128, NB, 512).transpose(2, 1, 0, 3)
        ).astype(f16)                                          # [NB,128,KT,512]

        def wslice(W):
            wt = W[cols, :].T                                  # [D, C]
            return np.ascontiguousarray(
                wt.reshape(KT, 128, C).transpose(1, 0, 2)
            ).astype(f16)                                      # [128, KT, C]

        wo_t = Wo[:, cols].T                                   # [C, D]
        wo = np.ascontiguousarray(
            wo_t.reshape(HL, 128, D).transpose(1, 0, 2)
        ).astype(f16)                                          # [128, HL, D]

        in_maps.append({
            "xt": xt,
            "wq": wslice(Wq),
            "wk": wslice(Wk),
            "wv": wslice(Wv),
            "wo": wo,
            "cos": cos.astype(f16),
            "sin": sin_signed.astype(f16),
            "onem": np.ones((128, 128), dtype=f16),
        })
    return in_maps


def kernel(x, Wq, Wk, Wv, Wo):
    x = np.asarray(x, dtype=np.float32)
    Wq = np.asarray(Wq, dtype=np.float32)
    Wk = np.asarray(Wk, dtype=np.float32)
    Wv = np.asarray(Wv, dtype=np.float32)
    Wo = np.asarray(Wo, dtype=np.float32)

    if "nc" not in _CACHE:
        _CACHE["nc"] = _build_program()
    nc = _CACHE["nc"]

    in_maps = _make_in_maps(x, Wq, Wk, Wv, Wo)
    results = run_bass_kernel_spmd(
        nc, in_maps, core_ids=list(range(N_CORES))
    ).results

    out = np.zeros((B, N, D), dtype=np.float32)
    for c in range(N_CORES):
        out[c // 4] += results[c]["out"].astype(np.float32)
    return out
